# revision 7
# baseline (speedup 1.0000x reference)
"""Trainium2 Bass kernel: single-head attention encoder block.

Problem: x[4, 2048, 1024]; q/k/v projections, softmax attention, output
projection, layernorm.  8 NeuronCores, SPMD.

Sharding: core c handles batch b = c // 2 and query-half h = c % 2.
Each core receives its batch's x ROTATED along the sequence axis so that
the core's 1024 query rows always occupy rows 0:1024 (attention is
permutation-invariant over keys as long as K and V share an ordering, so
the rotation only permutes the reduction order).  This keeps the SPMD
program free of per-core constants.

Per-core dataflow (all matmuls in float32r = full PE rate, fp32 storage):
  xt   = x^T                      (PE transpose, [d partition, s free])
  V    = x @ Wv                   (spilled to a DRAM scratch tile)
  K^T  = Wk^T @ x                 ([k partition, s free], SBUF resident)
  Q^T  = Wq^T @ x[:1024]          ([k partition, q free], SBUF resident)
  S^T  = K Q^T                    ([s partition, q free] -> exp via ACT)
  den  = ones^T @ exp(S^T)        (column sums via PE broadcast-matmul)
  A^T  = exp(S^T) * (1/den)
  ctxT = (A V)^T                  (lhsT=V-tile, rhs=A^T; V streamed back)
  h    = ctx @ Wo                 ([q partition, d free])
  out  = layernorm(h) * gamma + beta

SBUF is managed as two LIFO stacks (left/right) so overlapping pool
lifetimes can nest: left holds const/xt/attn/Wo, right holds K^T/Q^T and
then ctxT/V-stream.  Peak ~181 KB/partition.
"""

from contextlib import ExitStack

import numpy as np

import concourse.bass as bass
import concourse.tile as tile
from concourse import bacc, mybir
from concourse.bass_utils import run_bass_kernel_spmd
from concourse.masks import make_identity

F32 = mybir.dt.float32
F32R = mybir.dt.float32r
AF = mybir.ActivationFunctionType
OP = mybir.AluOpType

B = 4
S = 2048
D = 1024
NQ = 1024  # queries per core
P = 128
DT = D // P   # 8 d-tiles
ST = S // P   # 16 s-tiles
KTN = D // P  # 8 k-tiles
QTN = NQ // P  # 8 q-tiles
NC = 512      # matmul free-dim chunk (one fp32 PSUM bank)
SCN = S // NC   # 4 s-chunks
QCN = NQ // NC  # 2 q-chunks
DCN = D // NC   # 2 d-chunks
N_CORES = 8
SCALE = 1.0 / np.sqrt(np.float32(D))  # 1/32
LN_EPS = 1e-5


def _r(ap):
    """float32r view of an fp32 AP (full-rate PE matmul, fp32 storage)."""
    return ap.bitcast(F32R)


def _emit(ctx: ExitStack, tc: tile.TileContext, io: dict):
    nc = tc.nc
    xb = io["xb"]
    wq = io["wq"]
    wk = io["wk"]
    wv = io["wv"]
    wo = io["wo"]
    gamma_b = io["gamma_b"]
    beta_b = io["beta_b"]
    out = io["out"]

    dram = ctx.enter_context(tc.tile_pool(name="dram", bufs=1, space="DRAM"))
    v_dram = dram.tile([S, D], F32R, tag="vscratch", name="v_dram")

    const = ctx.enter_context(tc.tile_pool(name="const", bufs=1, side="left"))
    identity = const.tile([P, P], F32, tag="identity")
    make_identity(nc, identity[:])
    ones_f = const.tile([P, P], F32, tag="ones_f")
    nc.vector.memset(ones_f[:], 1.0)
    ones = const.tile([P, P], F32R, tag="ones")
    nc.vector.tensor_copy(ones[:], ones_f[:])
    gamma_sb = const.tile([P, D], F32, tag="gamma")
    nc.sync.dma_start(gamma_sb[:], gamma_b[:])
    beta_sb = const.tile([P, D], F32, tag="beta")
    nc.sync.dma_start(beta_sb[:], beta_b[:])
    recip = const.tile([P, NQ], F32, tag="recip")
    eps_sb = const.tile([P, 1], F32, tag="eps")
    nc.vector.memset(eps_sb[:], LN_EPS)

    # PSUM: 8 banks total.  mm cycles 4, den 2, transpose 2.
    ps_mm = ctx.enter_context(tc.tile_pool(name="ps_mm", bufs=4, space="PSUM"))
    ps_den = ctx.enter_context(tc.tile_pool(name="ps_den", bufs=2, space="PSUM"))
    ps_tr = ctx.enter_context(tc.tile_pool(name="ps_tr", bufs=2, space="PSUM"))

    # ---- Phase T: xt = x^T (PE transpose, 128 tiles) ----
    xtb_pool = tc.alloc_tile_pool(name="xtb", bufs=1, side="left")
    xtb = [xtb_pool.tile([P, S], F32R, tag=f"xtb{d}", name=f"xtb{d}") for d in range(DT)]
    xrow_pool = tc.alloc_tile_pool(name="xrow", bufs=3, side="left")
    for st in range(ST):
        xr = xrow_pool.tile([P, D], F32, tag="xr", name=f"xr{st}")
        nc.sync.dma_start(xr[:], xb[st * P:(st + 1) * P, :])
        for d in range(DT):
            pt = ps_tr.tile([P, P], F32, tag="tr", name=f"ptT{st}_{d}")
            nc.tensor.transpose(pt[:], xr[:, d * P:(d + 1) * P], identity[:])
            nc.vector.tensor_copy(xtb[d][:, st * P:(st + 1) * P], pt[:])
    xrow_pool.release()

    # ---- Phase V: V = x @ Wv -> v_dram ----
    wv_pool = tc.alloc_tile_pool(name="wvp", bufs=1, side="left")
    wv_sb = [wv_pool.tile([P, D], F32R, tag=f"wv{d}", name=f"wv{d}") for d in range(DT)]
    for d in range(DT):
        nc.sync.dma_start(wv_sb[d][:], wv[d * P:(d + 1) * P, :])
    vstage_pool = tc.alloc_tile_pool(name="vstage", bufs=3, side="left")
    for vc in range(DCN):
        for st in range(ST):
            ps = ps_mm.tile([P, NC], F32, tag="mm", name=f"psV{vc}_{st}")
            for d in range(DT):
                nc.tensor.matmul(
                    ps[:],
                    xtb[d][:, st * P:(st + 1) * P],
                    wv_sb[d][:, vc * NC:(vc + 1) * NC],
                    start=(d == 0),
                    stop=(d == DT - 1),
                )
            vs = vstage_pool.tile([P, NC], F32R, tag="vs", name=f"vs{vc}_{st}")
            nc.vector.tensor_copy(vs[:], ps[:])
            nc.sync.dma_start(v_dram[st * P:(st + 1) * P, vc * NC:(vc + 1) * NC], vs[:])
    vstage_pool.release()
    wv_pool.release()

    # ---- Phase K: K^T = Wk^T @ x  ([k, s], resident, right side) ----
    kt_pool = tc.alloc_tile_pool(name="ktp", bufs=1, side="right")
    kt_sb = [kt_pool.tile([P, S], F32R, tag=f"kt{k}", name=f"kt{k}") for k in range(KTN)]
    wstream_pool = tc.alloc_tile_pool(name="wstream", bufs=16, side="left")
    for k in range(KTN):
        wk_t = []
        for d in range(DT):
            wt = wstream_pool.tile([P, P], F32R, tag="wkt", name=f"wk{k}_{d}")
            nc.sync.dma_start(wt[:], wk[d * P:(d + 1) * P, k * P:(k + 1) * P])
            wk_t.append(wt)
        for sc in range(SCN):
            ps = ps_mm.tile([P, NC], F32, tag="mm", name=f"psK{k}_{sc}")
            for d in range(DT):
                nc.tensor.matmul(
                    ps[:],
                    wk_t[d][:],
                    xtb[d][:, sc * NC:(sc + 1) * NC],
                    start=(d == 0),
                    stop=(d == DT - 1),
                )
            nc.vector.tensor_copy(kt_sb[k][:, sc * NC:(sc + 1) * NC], ps[:])

    # ---- Phase Q: Q^T = Wq^T @ x[:, :NQ]  ([k, q], resident, right side) ----
    qt_pool = tc.alloc_tile_pool(name="qtp", bufs=1, side="right")
    qt_sb = [qt_pool.tile([P, NQ], F32R, tag=f"qt{k}", name=f"qt{k}") for k in range(KTN)]
    for k in range(KTN):
        wq_t = []
        for d in range(DT):
            wt = wstream_pool.tile([P, P], F32R, tag="wkt", name=f"wq{k}_{d}")
            nc.sync.dma_start(wt[:], wq[d * P:(d + 1) * P, k * P:(k + 1) * P])
            wq_t.append(wt)
        for qc in range(QCN):
            ps = ps_mm.tile([P, NC], F32, tag="mm", name=f"psQ{k}_{qc}")
            for d in range(DT):
                nc.tensor.matmul(
                    ps[:],
                    wq_t[d][:],
                    xtb[d][:, qc * NC:(qc + 1) * NC],
                    start=(d == 0),
                    stop=(d == DT - 1),
                )
            nc.vector.tensor_copy(qt_sb[k][:, qc * NC:(qc + 1) * NC], ps[:])
    wstream_pool.release()
    xtb_pool.release()

    # ---- Phase S: scores^T, exp, denominators, normalize ----
    at_pool = tc.alloc_tile_pool(name="atp", bufs=1, side="left")
    at_sb = [at_pool.tile([P, NQ], F32R, tag=f"at{st}", name=f"at{st}") for st in range(ST)]
    den_pool = tc.alloc_tile_pool(name="denp", bufs=2, side="left")
    for qc in range(QCN):
        dsb = den_pool.tile([P, NC], F32, tag="densb", name=f"densb{qc}")
        nc.vector.memset(dsb[:], 0.0)
        for st in range(ST):
            ps = ps_mm.tile([P, NC], F32, tag="mm", name=f"psS{qc}_{st}")
            for k in range(KTN):
                nc.tensor.matmul(
                    ps[:],
                    kt_sb[k][:, st * P:(st + 1) * P],
                    qt_sb[k][:, qc * NC:(qc + 1) * NC],
                    start=(k == 0),
                    stop=(k == KTN - 1),
                )
            # attn = exp(scores / sqrt(dk)); max-subtraction is unnecessary
            # here (scores are O(1) by construction) and softmax is
            # shift-invariant, so this matches the reference.
            nc.scalar.activation(
                at_sb[st][:, qc * NC:(qc + 1) * NC], ps[:], AF.Exp, scale=float(SCALE)
            )
            nc.vector.tensor_tensor(
                dsb[:], dsb[:], at_sb[st][:, qc * NC:(qc + 1) * NC].bitcast(F32),
                OP.add,
            )
        # Column sums replicated to all 128 partitions: ones[128,128]^T @ dsb.
        dsr = den_pool.tile([P, NC], F32R, tag="densr", name=f"densr{qc}")
        nc.vector.tensor_copy(dsr[:], dsb[:])
        dps = ps_den.tile([P, NC], F32, tag="den", name=f"dps{qc}")
        nc.tensor.matmul(dps[:], ones[:], dsr[:], start=True, stop=True)
        nc.vector.reciprocal(recip[:, qc * NC:(qc + 1) * NC], dps[:])
        for st in range(ST):
            nc.vector.tensor_tensor(
                at_sb[st][:, qc * NC:(qc + 1) * NC],
                at_sb[st][:, qc * NC:(qc + 1) * NC].bitcast(F32),
                recip[:, qc * NC:(qc + 1) * NC],
                OP.mult,
            )
    den_pool.release()
    qt_pool.release()
    kt_pool.release()

    # ---- Phase C: ctxT = (A @ V)^T  ([v, q]), V streamed from DRAM ----
    # Wo is prefetched here (right side, below ctxT) so its DMA overlaps
    # the ctx matmuls instead of stalling phase O.
    wo_pool = tc.alloc_tile_pool(name="wop", bufs=1, side="right")
    wo_sb = [wo_pool.tile([P, D], F32R, tag=f"wo{v}", name=f"wo{v}") for v in range(DT)]
    for v in range(DT):
        nc.sync.dma_start(wo_sb[v][:], wo[v * P:(v + 1) * P, :])
    ctxT_pool = tc.alloc_tile_pool(name="ctxTp", bufs=1, side="right")
    ctxT = [ctxT_pool.tile([P, NQ], F32R, tag=f"cxT{v}", name=f"cxT{v}") for v in range(DT)]
    vsb_pool = tc.alloc_tile_pool(name="vsb", bufs=20, side="right")
    for vc in range(DCN):
        v_t = []
        for st in range(ST):
            vt = vsb_pool.tile([P, NC], F32R, tag="vsb", name=f"vsb{vc}_{st}")
            nc.sync.dma_start(vt[:], v_dram[st * P:(st + 1) * P, vc * NC:(vc + 1) * NC])
            v_t.append(vt)
        for j in range(4):  # v-tile within this 512-wide chunk
            vti = vc * 4 + j
            for qc in range(QCN):
                ps = ps_mm.tile([P, NC], F32, tag="mm", name=f"psC{vti}_{qc}")
                for st in range(ST):
                    nc.tensor.matmul(
                        ps[:],
                        v_t[st][:, j * P:(j + 1) * P],
                        at_sb[st][:, qc * NC:(qc + 1) * NC],
                        start=(st == 0),
                        stop=(st == ST - 1),
                    )
                nc.vector.tensor_copy(ctxT[vti][:, qc * NC:(qc + 1) * NC], ps[:])
    vsb_pool.release()
    at_pool.release()

    # ---- Phase O: h = ctx @ Wo, layernorm, store ----
    h_pool = tc.alloc_tile_pool(name="hp", bufs=2, side="left")
    o_pool = tc.alloc_tile_pool(name="op", bufs=2, side="left")
    stat_pool = tc.alloc_tile_pool(name="statp", bufs=4, side="left")
    BN_FMAX = nc.vector.BN_STATS_FMAX
    n_sub = (D + BN_FMAX - 1) // BN_FMAX
    sub = D // n_sub
    for qt in range(QTN):
        h = h_pool.tile([P, D], F32, tag="h", name=f"h{qt}")
        for dc in range(DCN):
            ps = ps_mm.tile([P, NC], F32, tag="mm", name=f"psO{qt}_{dc}")
            for v in range(DT):
                nc.tensor.matmul(
                    ps[:],
                    ctxT[v][:, qt * P:(qt + 1) * P],
                    wo_sb[v][:, dc * NC:(dc + 1) * NC],
                    start=(v == 0),
                    stop=(v == DT - 1),
                )
            nc.vector.tensor_copy(h[:, dc * NC:(dc + 1) * NC], ps[:])
        # LayerNorm over the free dim via bn_stats/bn_aggr.
        stats = stat_pool.tile(
            [P, n_sub, nc.vector.BN_STATS_DIM], F32, tag="bnstats", name=f"bnst{qt}"
        )
        for i in range(n_sub):
            nc.vector.bn_stats(out=stats[:, i, :], in_=h[:, i * sub:(i + 1) * sub])
        mv = stat_pool.tile([P, nc.vector.BN_AGGR_DIM], F32, tag="bnaggr", name=f"bnag{qt}")
        nc.vector.bn_aggr(out=mv[:], in_=stats[:])
        # rstd = 1/sqrt(var + eps)
        rstd = stat_pool.tile([P, 1], F32, tag="rstd", name=f"rstd{qt}")
        nc.scalar.activation(rstd[:], mv[:, 1:2], AF.Sqrt, bias=eps_sb[:], scale=1.0)
        nc.vector.reciprocal(rstd[:], rstd[:])
        o = o_pool.tile([P, D], F32, tag="o", name=f"o{qt}")
        nc.vector.tensor_scalar(
            out=o[:],
            in0=h[:],
            scalar1=mv[:, 0:1],
            scalar2=rstd[:],
            op0=OP.subtract,
            op1=OP.mult,
        )
        nc.vector.tensor_tensor(o[:], o[:], gamma_sb[:], OP.mult)
        nc.vector.tensor_tensor(o[:], o[:], beta_sb[:], OP.add)
        nc.sync.dma_start(out[qt * P:(qt + 1) * P, :], o[:])
    stat_pool.release()
    o_pool.release()
    h_pool.release()
    ctxT_pool.release()
    wo_pool.release()


_PROGS: dict = {}


def _build_program(n_iters: int = 1):
    if n_iters not in _PROGS:
        nc = bacc.Bacc(
            "TRN2",
            target_bir_lowering=False,
            debug=False,
            enable_asserts=False,
            num_devices=N_CORES,
        )
        io = {
            "xb": nc.dram_tensor("xb", [S, D], F32, kind="ExternalInput").ap(),
            "wq": nc.dram_tensor("wq", [D, D], F32R, kind="ExternalInput").ap(),
            "wk": nc.dram_tensor("wk", [D, D], F32R, kind="ExternalInput").ap(),
            "wv": nc.dram_tensor("wv", [D, D], F32R, kind="ExternalInput").ap(),
            "wo": nc.dram_tensor("wo", [D, D], F32R, kind="ExternalInput").ap(),
            "gamma_b": nc.dram_tensor("gamma_b", [P, D], F32, kind="ExternalInput").ap(),
            "beta_b": nc.dram_tensor("beta_b", [P, D], F32, kind="ExternalInput").ap(),
            "out": nc.dram_tensor("out", [NQ, D], F32, kind="ExternalOutput").ap(),
        }
        with tile.TileContext(nc) as tc:
            for _ in range(n_iters):
                with ExitStack() as ctx:
                    _emit(ctx, tc, io)
        nc.compile()
        _PROGS[n_iters] = nc
    return _PROGS[n_iters]


LAST_RESULTS = None


def kernel(x, Wq, Wk, Wv, Wo, ln2_gamma, ln2_beta):
    global LAST_RESULTS
    x = np.ascontiguousarray(np.asarray(x, dtype=np.float32))
    Wq = np.ascontiguousarray(np.asarray(Wq, dtype=np.float32))
    Wk = np.ascontiguousarray(np.asarray(Wk, dtype=np.float32))
    Wv = np.ascontiguousarray(np.asarray(Wv, dtype=np.float32))
    Wo = np.ascontiguousarray(np.asarray(Wo, dtype=np.float32))
    gamma_b = np.ascontiguousarray(
        np.broadcast_to(np.asarray(ln2_gamma, dtype=np.float32), (P, D))
    )
    beta_b = np.ascontiguousarray(
        np.broadcast_to(np.asarray(ln2_beta, dtype=np.float32), (P, D))
    )

    nc = _build_program()
    in_maps = []
    for c in range(N_CORES):
        b, h = c // 2, c % 2
        # Rotate so this core's query rows are rows 0:NQ.
        xb = np.ascontiguousarray(np.roll(x[b], -h * NQ, axis=0))
        in_maps.append(
            {
                "xb": xb,
                "wq": Wq,
                "wk": Wk,
                "wv": Wv,
                "wo": Wo,
                "gamma_b": gamma_b,
                "beta_b": beta_b,
            }
        )
    res = run_bass_kernel_spmd(nc, in_maps, list(range(N_CORES)))
    LAST_RESULTS = res
    out = np.empty((B, S, D), dtype=np.float32)
    for c in range(N_CORES):
        b, h = c // 2, c % 2
        out[b, h * NQ:(h + 1) * NQ] = res.results[c]["out"]
    return out


# revision 8
# speedup vs baseline: 1.0213x; 1.0213x over previous
"""Trainium2 Bass kernel: single-head attention encoder block.

Problem: x[4, 2048, 1024]; q/k/v projections, softmax attention, output
projection, layernorm.  8 NeuronCores, SPMD.

Sharding: core c handles batch b = c // 2 and query-half h = c % 2.
Each core receives its batch's x ROTATED along the sequence axis so that
the core's 1024 query rows always occupy rows 0:1024 (attention is
permutation-invariant over keys as long as K and V share an ordering, so
the rotation only permutes the reduction order).  This keeps the SPMD
program free of per-core constants.

Per-core dataflow (all matmuls in float32r = full PE rate, fp32 storage).
The value path uses associativity:  ctx = A @ (x @ Wv) = (A @ x) @ Wv,
which removes the V projection for the full sequence AND the scratch
round-trip; the Z^T = x^T @ A^T intermediate takes x tiles straight from
DRAM as the stationary operand (no transpose needed), and comes out in
exactly the layout the Wv/Wo projections want.

  xt    = x^T                      (PE transpose, [d partition, s free])
  K^T   = Wk^T @ x                 ([k partition, s free], SBUF resident)
  Q^T   = Wq^T @ x[:1024]          ([k partition, q free], SBUF resident)
  S^T   = K Q^T                    ([s partition, q free] -> exp via ACT)
  den   = ones^T @ exp(S^T)        (column sums via PE broadcast-matmul)
  Z^T   = x^T @ exp(S^T)           (lhsT = x tiles from DRAM, [d, q])
  ctxT  = (Wv^T @ Z^T) * 1/den     ([v, q]; normalization fused in copy)
  h     = ctx @ Wo                 ([q partition, d free])
  out   = layernorm(h) * gamma + beta

SBUF is two LIFO stacks (left/right) so overlapping pool lifetimes nest.
Peak ~185 KB/partition.
"""

from contextlib import ExitStack

import numpy as np

import concourse.bass as bass
import concourse.tile as tile
from concourse import bacc, mybir
from concourse.bass_utils import run_bass_kernel_spmd
from concourse.masks import make_identity

F32 = mybir.dt.float32
F32R = mybir.dt.float32r
AF = mybir.ActivationFunctionType
OP = mybir.AluOpType

B = 4
S = 2048
D = 1024
NQ = 1024  # queries per core
P = 128
DT = D // P   # 8 d-tiles
ST = S // P   # 16 s-tiles
KTN = D // P  # 8 k-tiles
QTN = NQ // P  # 8 q-tiles
NC = 512      # matmul free-dim chunk (one fp32 PSUM bank)
SCN = S // NC   # 4 s-chunks
QCN = NQ // NC  # 2 q-chunks
DCN = D // NC   # 2 d-chunks
N_CORES = 8
SCALE = 1.0 / np.sqrt(np.float32(D))  # 1/32
LN_EPS = 1e-5


def _f32(ap):
    """fp32 view of an f32r AP for DVE/ACT readers (same IEEE bits)."""
    return ap.bitcast(F32)


def _emit(ctx: ExitStack, tc: tile.TileContext, io: dict):
    nc = tc.nc
    xb = io["xb"]          # [S, D] f32r
    wq = io["wq"]          # [D, D] f32r
    wk = io["wk"]
    wv = io["wv"]
    wo = io["wo"]
    gamma_b = io["gamma_b"]  # [P, D] f32
    beta_b = io["beta_b"]
    out = io["out"]        # [NQ, D] f32

    const = ctx.enter_context(tc.tile_pool(name="const", bufs=1, side="left"))
    identity_f = const.tile([P, P], F32, tag="identity_f")
    make_identity(nc, identity_f[:])
    identity = const.tile([P, P], F32R, tag="identity")
    nc.vector.tensor_copy(identity[:], identity_f[:])
    ones_f = const.tile([P, P], F32, tag="ones_f")
    nc.vector.memset(ones_f[:], 1.0)
    ones = const.tile([P, P], F32R, tag="ones")
    nc.vector.tensor_copy(ones[:], ones_f[:])
    gamma_sb = const.tile([P, D], F32, tag="gamma")
    nc.sync.dma_start(gamma_sb[:], gamma_b[:])
    beta_sb = const.tile([P, D], F32, tag="beta")
    nc.sync.dma_start(beta_sb[:], beta_b[:])
    recip = const.tile([P, NQ], F32, tag="recip")
    eps_sb = const.tile([P, 1], F32, tag="eps")
    nc.vector.memset(eps_sb[:], LN_EPS)

    # PSUM: 8 banks.  mm cycles 4, den 2, transpose 2.
    ps_mm = ctx.enter_context(tc.tile_pool(name="ps_mm", bufs=4, space="PSUM"))
    ps_den = ctx.enter_context(tc.tile_pool(name="ps_den", bufs=2, space="PSUM"))
    ps_tr = ctx.enter_context(tc.tile_pool(name="ps_tr", bufs=2, space="PSUM"))

    # ---- Phase T: xt = x^T (PE transpose, 128 tiles) ----
    xtb_pool = tc.alloc_tile_pool(name="xtb", bufs=1, side="left")
    xtb = [xtb_pool.tile([P, S], F32R, tag=f"xtb{d}", name=f"xtb{d}") for d in range(DT)]
    xrow_pool = tc.alloc_tile_pool(name="xrow", bufs=3, side="left")
    for st in range(ST):
        xr = xrow_pool.tile([P, D], F32R, tag="xr", name=f"xr{st}")
        nc.sync.dma_start(xr[:], xb[st * P:(st + 1) * P, :])
        for d in range(DT):
            pt = ps_tr.tile([P, P], F32R, tag="tr", name=f"ptT{st}_{d}")
            nc.tensor.transpose(pt[:], xr[:, d * P:(d + 1) * P], identity[:])
            nc.vector.tensor_copy(xtb[d][:, st * P:(st + 1) * P], _f32(pt[:]))
    xrow_pool.release()

    # ---- Phase K: K^T = Wk^T @ x  ([k, s], resident, right side) ----
    kt_pool = tc.alloc_tile_pool(name="ktp", bufs=1, side="right")
    kt_sb = [kt_pool.tile([P, S], F32R, tag=f"kt{k}", name=f"kt{k}") for k in range(KTN)]
    wstream_pool = tc.alloc_tile_pool(name="wstream", bufs=16, side="left")
    for k in range(KTN):
        wk_t = []
        for d in range(DT):
            wt = wstream_pool.tile([P, P], F32R, tag="wkt", name=f"wk{k}_{d}")
            nc.sync.dma_start(wt[:], wk[d * P:(d + 1) * P, k * P:(k + 1) * P])
            wk_t.append(wt)
        for sc in range(SCN):
            ps = ps_mm.tile([P, NC], F32, tag="mm", name=f"psK{k}_{sc}")
            for d in range(DT):
                nc.tensor.matmul(
                    ps[:],
                    wk_t[d][:],
                    xtb[d][:, sc * NC:(sc + 1) * NC],
                    start=(d == 0),
                    stop=(d == DT - 1),
                )
            nc.vector.tensor_copy(kt_sb[k][:, sc * NC:(sc + 1) * NC], ps[:])

    # ---- Phase Q: Q^T = Wq^T @ x[:, :NQ]  ([k, q], resident, right side) ----
    qt_pool = tc.alloc_tile_pool(name="qtp", bufs=1, side="right")
    qt_sb = [qt_pool.tile([P, NQ], F32R, tag=f"qt{k}", name=f"qt{k}") for k in range(KTN)]
    for k in range(KTN):
        wq_t = []
        for d in range(DT):
            wt = wstream_pool.tile([P, P], F32R, tag="wkt", name=f"wq{k}_{d}")
            nc.sync.dma_start(wt[:], wq[d * P:(d + 1) * P, k * P:(k + 1) * P])
            wq_t.append(wt)
        for qc in range(QCN):
            ps = ps_mm.tile([P, NC], F32, tag="mm", name=f"psQ{k}_{qc}")
            for d in range(DT):
                nc.tensor.matmul(
                    ps[:],
                    wq_t[d][:],
                    xtb[d][:, qc * NC:(qc + 1) * NC],
                    start=(d == 0),
                    stop=(d == DT - 1),
                )
            nc.vector.tensor_copy(qt_sb[k][:, qc * NC:(qc + 1) * NC], ps[:])
    wstream_pool.release()
    xtb_pool.release()

    # ---- Phase S: scores^T -> exp (UNNORMALIZED), denominators ----
    at_pool = tc.alloc_tile_pool(name="atp", bufs=1, side="left")
    at_sb = [at_pool.tile([P, NQ], F32R, tag=f"at{st}", name=f"at{st}") for st in range(ST)]
    den_pool = tc.alloc_tile_pool(name="denp", bufs=2, side="left")
    for qc in range(QCN):
        dsb = den_pool.tile([P, NC], F32, tag="densb", name=f"densb{qc}")
        nc.vector.memset(dsb[:], 0.0)
        for st in range(ST):
            ps = ps_mm.tile([P, NC], F32, tag="mm", name=f"psS{qc}_{st}")
            for k in range(KTN):
                nc.tensor.matmul(
                    ps[:],
                    kt_sb[k][:, st * P:(st + 1) * P],
                    qt_sb[k][:, qc * NC:(qc + 1) * NC],
                    start=(k == 0),
                    stop=(k == KTN - 1),
                )
            # attn = exp(scores / sqrt(dk)); max-subtraction is unnecessary
            # here (scores are O(1) by construction) and softmax is
            # shift-invariant, so this matches the reference.
            nc.scalar.activation(
                at_sb[st][:, qc * NC:(qc + 1) * NC], ps[:], AF.Exp, scale=float(SCALE)
            )
            nc.vector.tensor_tensor(
                dsb[:], dsb[:], _f32(at_sb[st][:, qc * NC:(qc + 1) * NC]), OP.add
            )
        # Column sums replicated to all 128 partitions: ones[128,128]^T @ dsb.
        dsr = den_pool.tile([P, NC], F32R, tag="densr", name=f"densr{qc}")
        nc.vector.tensor_copy(dsr[:], dsb[:])
        dps = ps_den.tile([P, NC], F32, tag="den", name=f"dps{qc}")
        nc.tensor.matmul(dps[:], ones[:], dsr[:], start=True, stop=True)
        nc.vector.reciprocal(recip[:, qc * NC:(qc + 1) * NC], dps[:])
    den_pool.release()
    qt_pool.release()
    kt_pool.release()

    # ---- Phase C1: Z^T = x^T @ exp(S^T)  ([d, q]; x tiles from DRAM) ----
    # Wo and Wv prefetch here (right side, below ZT) so their DMA overlaps
    # the C1/C2 matmuls.
    wo_pool = tc.alloc_tile_pool(name="wop", bufs=1, side="right")
    wo_sb = [wo_pool.tile([P, D], F32R, tag=f"wo{v}", name=f"wo{v}") for v in range(DT)]
    for v in range(DT):
        nc.sync.dma_start(wo_sb[v][:], wo[v * P:(v + 1) * P, :])
    wv_pool = tc.alloc_tile_pool(name="wvp", bufs=1, side="right")
    wv_sb = [wv_pool.tile([P, D], F32R, tag=f"wv{d}", name=f"wv{d}") for d in range(DT)]
    for d in range(DT):
        nc.sync.dma_start(wv_sb[d][:], wv[d * P:(d + 1) * P, :])
    zt_pool = tc.alloc_tile_pool(name="ztp", bufs=1, side="right")
    zt_sb = [zt_pool.tile([P, NQ], F32R, tag=f"zt{d}", name=f"zt{d}") for d in range(DT)]
    xcol_pool = tc.alloc_tile_pool(name="xcol", bufs=24, side="right")
    for d in range(DT):
        xc = []
        for st in range(ST):
            t = xcol_pool.tile([P, P], F32R, tag="xc", name=f"xc{d}_{st}")
            nc.sync.dma_start(t[:], xb[st * P:(st + 1) * P, d * P:(d + 1) * P])
            xc.append(t)
        for qc in range(QCN):
            ps = ps_mm.tile([P, NC], F32, tag="mm", name=f"psZ{d}_{qc}")
            for st in range(ST):
                nc.tensor.matmul(
                    ps[:],
                    xc[st][:],
                    at_sb[st][:, qc * NC:(qc + 1) * NC],
                    start=(st == 0),
                    stop=(st == ST - 1),
                )
            nc.vector.tensor_copy(zt_sb[d][:, qc * NC:(qc + 1) * NC], ps[:])
    xcol_pool.release()
    at_pool.release()

    # ---- Phase C2: ctxT = (Wv^T @ Z^T) * 1/den  ([v, q]) ----
    ctxT_pool = tc.alloc_tile_pool(name="ctxTp", bufs=1, side="left")
    ctxT = [ctxT_pool.tile([P, NQ], F32R, tag=f"cxT{v}", name=f"cxT{v}") for v in range(DT)]
    for vt in range(DT):
        for qc in range(QCN):
            ps = ps_mm.tile([P, NC], F32, tag="mm", name=f"psC{vt}_{qc}")
            for d in range(DT):
                nc.tensor.matmul(
                    ps[:],
                    wv_sb[d][:, vt * P:(vt + 1) * P],
                    zt_sb[d][:, qc * NC:(qc + 1) * NC],
                    start=(d == 0),
                    stop=(d == DT - 1),
                )
            # normalization fused into the PSUM drain
            nc.vector.tensor_tensor(
                ctxT[vt][:, qc * NC:(qc + 1) * NC],
                ps[:],
                recip[:, qc * NC:(qc + 1) * NC],
                OP.mult,
            )
    zt_pool.release()

    # ---- Phase O: h = ctx @ Wo, layernorm, store ----
    h_pool = tc.alloc_tile_pool(name="hp", bufs=2, side="left")
    o_pool = tc.alloc_tile_pool(name="op", bufs=2, side="left")
    stat_pool = tc.alloc_tile_pool(name="statp", bufs=4, side="left")
    BN_FMAX = nc.vector.BN_STATS_FMAX
    n_sub = (D + BN_FMAX - 1) // BN_FMAX
    sub = D // n_sub
    for qt in range(QTN):
        h = h_pool.tile([P, D], F32, tag="h", name=f"h{qt}")
        for dc in range(DCN):
            ps = ps_mm.tile([P, NC], F32, tag="mm", name=f"psO{qt}_{dc}")
            for v in range(DT):
                nc.tensor.matmul(
                    ps[:],
                    ctxT[v][:, qt * P:(qt + 1) * P],
                    wo_sb[v][:, dc * NC:(dc + 1) * NC],
                    start=(v == 0),
                    stop=(v == DT - 1),
                )
            nc.vector.tensor_copy(h[:, dc * NC:(dc + 1) * NC], ps[:])
        # LayerNorm over the free dim via bn_stats/bn_aggr.
        stats = stat_pool.tile(
            [P, n_sub, nc.vector.BN_STATS_DIM], F32, tag="bnstats", name=f"bnst{qt}"
        )
        for i in range(n_sub):
            nc.vector.bn_stats(out=stats[:, i, :], in_=h[:, i * sub:(i + 1) * sub])
        mv = stat_pool.tile([P, nc.vector.BN_AGGR_DIM], F32, tag="bnaggr", name=f"bnag{qt}")
        nc.vector.bn_aggr(out=mv[:], in_=stats[:])
        # rstd = 1/sqrt(var + eps)
        rstd = stat_pool.tile([P, 1], F32, tag="rstd", name=f"rstd{qt}")
        nc.scalar.activation(rstd[:], mv[:, 1:2], AF.Sqrt, bias=eps_sb[:], scale=1.0)
        nc.vector.reciprocal(rstd[:], rstd[:])
        o = o_pool.tile([P, D], F32, tag="o", name=f"o{qt}")
        nc.vector.tensor_scalar(
            out=o[:],
            in0=h[:],
            scalar1=mv[:, 0:1],
            scalar2=rstd[:],
            op0=OP.subtract,
            op1=OP.mult,
        )
        nc.vector.tensor_tensor(o[:], o[:], gamma_sb[:], OP.mult)
        nc.vector.tensor_tensor(o[:], o[:], beta_sb[:], OP.add)
        nc.sync.dma_start(out[qt * P:(qt + 1) * P, :], o[:])
    stat_pool.release()
    o_pool.release()
    h_pool.release()
    ctxT_pool.release()
    wv_pool.release()
    wo_pool.release()


_PROGS: dict = {}


def _build_program(n_iters: int = 1):
    if n_iters not in _PROGS:
        nc = bacc.Bacc(
            "TRN2",
            target_bir_lowering=False,
            debug=False,
            enable_asserts=False,
            num_devices=N_CORES,
        )
        io = {
            "xb": nc.dram_tensor("xb", [S, D], F32R, kind="ExternalInput").ap(),
            "wq": nc.dram_tensor("wq", [D, D], F32R, kind="ExternalInput").ap(),
            "wk": nc.dram_tensor("wk", [D, D], F32R, kind="ExternalInput").ap(),
            "wv": nc.dram_tensor("wv", [D, D], F32R, kind="ExternalInput").ap(),
            "wo": nc.dram_tensor("wo", [D, D], F32R, kind="ExternalInput").ap(),
            "gamma_b": nc.dram_tensor("gamma_b", [P, D], F32, kind="ExternalInput").ap(),
            "beta_b": nc.dram_tensor("beta_b", [P, D], F32, kind="ExternalInput").ap(),
            "out": nc.dram_tensor("out", [NQ, D], F32, kind="ExternalOutput").ap(),
        }
        with tile.TileContext(nc) as tc:
            for _ in range(n_iters):
                with ExitStack() as ctx:
                    _emit(ctx, tc, io)
        nc.compile()
        _PROGS[n_iters] = nc
    return _PROGS[n_iters]


LAST_RESULTS = None


def kernel(x, Wq, Wk, Wv, Wo, ln2_gamma, ln2_beta):
    global LAST_RESULTS
    x = np.ascontiguousarray(np.asarray(x, dtype=np.float32))
    Wq = np.ascontiguousarray(np.asarray(Wq, dtype=np.float32))
    Wk = np.ascontiguousarray(np.asarray(Wk, dtype=np.float32))
    Wv = np.ascontiguousarray(np.asarray(Wv, dtype=np.float32))
    Wo = np.ascontiguousarray(np.asarray(Wo, dtype=np.float32))
    gamma_b = np.ascontiguousarray(
        np.broadcast_to(np.asarray(ln2_gamma, dtype=np.float32), (P, D))
    )
    beta_b = np.ascontiguousarray(
        np.broadcast_to(np.asarray(ln2_beta, dtype=np.float32), (P, D))
    )

    nc = _build_program()
    in_maps = []
    for c in range(N_CORES):
        b, h = c // 2, c % 2
        # Rotate so this core's query rows are rows 0:NQ.
        xb = np.ascontiguousarray(np.roll(x[b], -h * NQ, axis=0))
        in_maps.append(
            {
                "xb": xb,
                "wq": Wq,
                "wk": Wk,
                "wv": Wv,
                "wo": Wo,
                "gamma_b": gamma_b,
                "beta_b": beta_b,
            }
        )
    res = run_bass_kernel_spmd(nc, in_maps, list(range(N_CORES)))
    LAST_RESULTS = res
    out = np.empty((B, S, D), dtype=np.float32)
    for c in range(N_CORES):
        b, h = c // 2, c % 2
        out[b, h * NQ:(h + 1) * NQ] = res.results[c]["out"]
    return out


# revision 9
# speedup vs baseline: 1.0278x; 1.0064x over previous
"""Trainium2 Bass kernel: single-head attention encoder block.

Problem: x[4, 2048, 1024]; q/k/v projections, softmax attention, output
projection, layernorm.  8 NeuronCores, SPMD.

Sharding: core c handles batch b = c // 2 and query-half h = c % 2.
Each core receives its batch's x ROTATED along the sequence axis so that
the core's 1024 query rows always occupy rows 0:1024 (attention is
permutation-invariant over keys as long as K and V share an ordering, so
the rotation only permutes the reduction order).  This keeps the SPMD
program free of per-core constants.

Per-core dataflow (all matmuls in float32r = full PE rate, fp32 storage).
The value path uses associativity:  ctx = A @ (x @ Wv) = (A @ x) @ Wv,
which removes the V projection for the full sequence AND the scratch
round-trip; the Z^T = x^T @ A^T intermediate takes x tiles straight from
DRAM as the stationary operand (no transpose needed), and comes out in
exactly the layout the Wv/Wo projections want.

  xt    = x^T                      (PE transpose, [d partition, s free])
  K^T   = Wk^T @ x                 ([k partition, s free], SBUF resident)
  Q^T   = Wq^T @ x[:1024]          ([k partition, q free], SBUF resident)
  S^T   = K Q^T                    ([s partition, q free] -> exp via ACT)
  den   = ones^T @ exp(S^T)        (column sums via PE broadcast-matmul)
  Z^T   = x^T @ exp(S^T)           (lhsT = x tiles from DRAM, [d, q])
  ctxT  = (Wv^T @ Z^T) * 1/den     ([v, q]; normalization fused in copy)
  h     = ctx @ Wo                 ([q partition, d free])
  out   = layernorm(h) * gamma + beta

SBUF is two LIFO stacks (left/right) so overlapping pool lifetimes nest.
Peak ~185 KB/partition.
"""

from contextlib import ExitStack

import numpy as np

import concourse.bass as bass
import concourse.tile as tile
from concourse import bacc, mybir
from concourse.bass_utils import run_bass_kernel_spmd
from concourse.masks import make_identity

F32 = mybir.dt.float32
F32R = mybir.dt.float32r
AF = mybir.ActivationFunctionType
OP = mybir.AluOpType

B = 4
S = 2048
D = 1024
NQ = 1024  # queries per core
P = 128
DT = D // P   # 8 d-tiles
ST = S // P   # 16 s-tiles
KTN = D // P  # 8 k-tiles
QTN = NQ // P  # 8 q-tiles
NC = 512      # matmul free-dim chunk (one fp32 PSUM bank)
SCN = S // NC   # 4 s-chunks
QCN = NQ // NC  # 2 q-chunks
DCN = D // NC   # 2 d-chunks
N_CORES = 8
SCALE = 1.0 / np.sqrt(np.float32(D))  # 1/32
LN_EPS = 1e-5


def _f32(ap):
    """fp32 view of an f32r AP for DVE/ACT readers (same IEEE bits)."""
    return ap.bitcast(F32)


def _emit(ctx: ExitStack, tc: tile.TileContext, io: dict):
    nc = tc.nc
    xb = io["xb"]          # [S, D] f32r
    wq = io["wq"]          # [D, D] f32r
    wk = io["wk"]
    wv = io["wv"]
    wo = io["wo"]
    gamma_b = io["gamma_b"]  # [P, D] f32
    beta_b = io["beta_b"]
    out = io["out"]        # [NQ, D] f32

    const = ctx.enter_context(tc.tile_pool(name="const", bufs=1, side="left"))
    identity_f = const.tile([P, P], F32, tag="identity_f")
    make_identity(nc, identity_f[:])
    identity = const.tile([P, P], F32R, tag="identity")
    nc.vector.tensor_copy(identity[:], identity_f[:])
    ones_f = const.tile([P, P], F32, tag="ones_f")
    nc.vector.memset(ones_f[:], 1.0)
    ones = const.tile([P, P], F32R, tag="ones")
    nc.vector.tensor_copy(ones[:], ones_f[:])
    recip = const.tile([P, NQ], F32, tag="recip")
    eps_sb = const.tile([P, 1], F32, tag="eps")
    nc.vector.memset(eps_sb[:], LN_EPS)

    # PSUM: 8 banks.  mm cycles 4, den 2, transpose 2.
    ps_mm = ctx.enter_context(tc.tile_pool(name="ps_mm", bufs=4, space="PSUM"))
    ps_den = ctx.enter_context(tc.tile_pool(name="ps_den", bufs=2, space="PSUM"))
    ps_tr = ctx.enter_context(tc.tile_pool(name="ps_tr", bufs=2, space="PSUM"))

    # ---- Phases T+K interleaved ----
    # xcol is pre-reserved at the BOTTOM of the right stack: its 8MB of x
    # column tiles (consumed in C1) can then stream during K/Q/S with no
    # released-zone dependency on kt/qt.
    xcol_pool = tc.alloc_tile_pool(name="xcol", bufs=24, side="right")
    kt_pool = tc.alloc_tile_pool(name="ktp", bufs=1, side="right")
    kt_sb = [kt_pool.tile([P, S], F32R, tag=f"kt{k}", name=f"kt{k}") for k in range(KTN)]
    xtb_pool = tc.alloc_tile_pool(name="xtb", bufs=1, side="left")
    xtb = [xtb_pool.tile([P, S], F32R, tag=f"xtb{d}", name=f"xtb{d}") for d in range(DT)]
    wk_pool = tc.alloc_tile_pool(name="wkp", bufs=1, side="left")
    xrow_pool = tc.alloc_tile_pool(name="xrow", bufs=2, side="left")
    wk_t: dict = {}

    def _transpose_chunk(sc):
        for st in range(4 * sc, 4 * sc + 4):
            xr = xrow_pool.tile([P, D], F32R, tag="xr", name=f"xr{st}")
            nc.sync.dma_start(xr[:], xb[st * P:(st + 1) * P, :])
            for d in range(DT):
                pt = ps_tr.tile([P, P], F32R, tag="tr", name=f"ptT{st}_{d}")
                nc.tensor.transpose(pt[:], xr[:, d * P:(d + 1) * P], identity[:])
                nc.vector.tensor_copy(xtb[d][:, st * P:(st + 1) * P], _f32(pt[:]))

    for sc in range(SCN):
        _transpose_chunk(sc)
        for k in range(KTN):
            if sc == 0:
                wk_t[k] = []
                for d in range(DT):
                    wt = wk_pool.tile([P, P], F32R, tag=f"wk{k}_{d}", name=f"wk{k}_{d}")
                    nc.sync.dma_start(wt[:], wk[d * P:(d + 1) * P, k * P:(k + 1) * P])
                    wk_t[k].append(wt)
            ps = ps_mm.tile([P, NC], F32, tag="mm", name=f"psK{k}_{sc}")
            for d in range(DT):
                nc.tensor.matmul(
                    ps[:],
                    wk_t[k][d][:],
                    xtb[d][:, sc * NC:(sc + 1) * NC],
                    start=(d == 0),
                    stop=(d == DT - 1),
                )
            nc.vector.tensor_copy(kt_sb[k][:, sc * NC:(sc + 1) * NC], ps[:])
    xrow_pool.release()
    wk_pool.release()

    # ---- Phase Q: Q^T = Wq^T @ x[:, :NQ]  ([k, q], resident, right side) ----
    qt_pool = tc.alloc_tile_pool(name="qtp", bufs=1, side="right")
    qt_sb = [qt_pool.tile([P, NQ], F32R, tag=f"qt{k}", name=f"qt{k}") for k in range(KTN)]
    wstream_pool = tc.alloc_tile_pool(name="wstream", bufs=16, side="left")
    for k in range(KTN):
        wq_t = []
        for d in range(DT):
            wt = wstream_pool.tile([P, P], F32R, tag="wkt", name=f"wq{k}_{d}")
            nc.sync.dma_start(wt[:], wq[d * P:(d + 1) * P, k * P:(k + 1) * P])
            wq_t.append(wt)
        for qc in range(QCN):
            ps = ps_mm.tile([P, NC], F32, tag="mm", name=f"psQ{k}_{qc}")
            for d in range(DT):
                nc.tensor.matmul(
                    ps[:],
                    wq_t[d][:],
                    xtb[d][:, qc * NC:(qc + 1) * NC],
                    start=(d == 0),
                    stop=(d == DT - 1),
                )
            nc.vector.tensor_copy(qt_sb[k][:, qc * NC:(qc + 1) * NC], ps[:])
    wstream_pool.release()
    xtb_pool.release()

    # ---- Phase S: scores^T -> exp (UNNORMALIZED), denominators ----
    at_pool = tc.alloc_tile_pool(name="atp", bufs=1, side="left")
    at_sb = [at_pool.tile([P, NQ], F32R, tag=f"at{st}", name=f"at{st}") for st in range(ST)]
    den_pool = tc.alloc_tile_pool(name="denp", bufs=2, side="left")
    for qc in range(QCN):
        dsb = den_pool.tile([P, NC], F32, tag="densb", name=f"densb{qc}")
        nc.vector.memset(dsb[:], 0.0)
        for st in range(ST):
            ps = ps_mm.tile([P, NC], F32, tag="mm", name=f"psS{qc}_{st}")
            for k in range(KTN):
                nc.tensor.matmul(
                    ps[:],
                    kt_sb[k][:, st * P:(st + 1) * P],
                    qt_sb[k][:, qc * NC:(qc + 1) * NC],
                    start=(k == 0),
                    stop=(k == KTN - 1),
                )
            # attn = exp(scores / sqrt(dk)); max-subtraction is unnecessary
            # here (scores are O(1) by construction) and softmax is
            # shift-invariant, so this matches the reference.
            nc.scalar.activation(
                at_sb[st][:, qc * NC:(qc + 1) * NC], ps[:], AF.Exp, scale=float(SCALE)
            )
            nc.vector.tensor_tensor(
                dsb[:], dsb[:], _f32(at_sb[st][:, qc * NC:(qc + 1) * NC]), OP.add
            )
        # Column sums replicated to all 128 partitions: ones[128,128]^T @ dsb.
        dsr = den_pool.tile([P, NC], F32R, tag="densr", name=f"densr{qc}")
        nc.vector.tensor_copy(dsr[:], dsb[:])
        dps = ps_den.tile([P, NC], F32, tag="den", name=f"dps{qc}")
        nc.tensor.matmul(dps[:], ones[:], dsr[:], start=True, stop=True)
        nc.vector.reciprocal(recip[:, qc * NC:(qc + 1) * NC], dps[:])
    den_pool.release()
    qt_pool.release()
    kt_pool.release()

    # ---- Phase C1: Z^T = x^T @ exp(S^T)  ([d, q]; x tiles from DRAM) ----
    # Wv/Wo prefetch (right side, on the kt/qt zone): their DMA overlaps
    # the C1 matmuls; wv lands first (needed at C2, wo only at O).
    wv_pool = tc.alloc_tile_pool(name="wvp", bufs=1, side="right")
    wv_sb = [wv_pool.tile([P, D], F32R, tag=f"wv{d}", name=f"wv{d}") for d in range(DT)]
    for d in range(DT):
        nc.sync.dma_start(wv_sb[d][:], wv[d * P:(d + 1) * P, :])
    wo_pool = tc.alloc_tile_pool(name="wop", bufs=1, side="right")
    wo_sb = [wo_pool.tile([P, D], F32R, tag=f"wo{v}", name=f"wo{v}") for v in range(DT)]
    for v in range(DT):
        nc.sync.dma_start(wo_sb[v][:], wo[v * P:(v + 1) * P, :])
    zt_pool = tc.alloc_tile_pool(name="ztp", bufs=1, side="right")
    zt_sb = [zt_pool.tile([P, NQ], F32R, tag=f"zt{d}", name=f"zt{d}") for d in range(DT)]
    for d in range(DT):
        xc = []
        for st in range(ST):
            t = xcol_pool.tile([P, P], F32R, tag="xc", name=f"xc{d}_{st}")
            nc.sync.dma_start(t[:], xb[st * P:(st + 1) * P, d * P:(d + 1) * P])
            xc.append(t)
        for qc in range(QCN):
            ps = ps_mm.tile([P, NC], F32, tag="mm", name=f"psZ{d}_{qc}")
            for st in range(ST):
                nc.tensor.matmul(
                    ps[:],
                    xc[st][:],
                    at_sb[st][:, qc * NC:(qc + 1) * NC],
                    start=(st == 0),
                    stop=(st == ST - 1),
                )
            nc.vector.tensor_copy(zt_sb[d][:, qc * NC:(qc + 1) * NC], ps[:])
    at_pool.release()

    # ---- Phase C2: ctxT = (Wv^T @ Z^T) * 1/den  ([v, q]) ----
    gb_pool = tc.alloc_tile_pool(name="gbp", bufs=1, side="left")
    gamma_sb = gb_pool.tile([P, D], F32, tag="gamma", name="gamma_sb")
    nc.sync.dma_start(gamma_sb[:], gamma_b[:])
    beta_sb = gb_pool.tile([P, D], F32, tag="beta", name="beta_sb")
    nc.sync.dma_start(beta_sb[:], beta_b[:])
    ctxT_pool = tc.alloc_tile_pool(name="ctxTp", bufs=1, side="left")
    ctxT = [ctxT_pool.tile([P, NQ], F32R, tag=f"cxT{v}", name=f"cxT{v}") for v in range(DT)]
    for vt in range(DT):
        for qc in range(QCN):
            ps = ps_mm.tile([P, NC], F32, tag="mm", name=f"psC{vt}_{qc}")
            for d in range(DT):
                nc.tensor.matmul(
                    ps[:],
                    wv_sb[d][:, vt * P:(vt + 1) * P],
                    zt_sb[d][:, qc * NC:(qc + 1) * NC],
                    start=(d == 0),
                    stop=(d == DT - 1),
                )
            # normalization fused into the PSUM drain
            nc.vector.tensor_tensor(
                ctxT[vt][:, qc * NC:(qc + 1) * NC],
                ps[:],
                recip[:, qc * NC:(qc + 1) * NC],
                OP.mult,
            )
    zt_pool.release()

    # ---- Phase O: h = ctx @ Wo, layernorm, store ----
    h_pool = tc.alloc_tile_pool(name="hp", bufs=2, side="left")
    o_pool = tc.alloc_tile_pool(name="op", bufs=2, side="left")
    stat_pool = tc.alloc_tile_pool(name="statp", bufs=4, side="left")
    BN_FMAX = nc.vector.BN_STATS_FMAX
    n_sub = (D + BN_FMAX - 1) // BN_FMAX
    sub = D // n_sub
    for qt in range(QTN):
        h = h_pool.tile([P, D], F32, tag="h", name=f"h{qt}")
        for dc in range(DCN):
            ps = ps_mm.tile([P, NC], F32, tag="mm", name=f"psO{qt}_{dc}")
            for v in range(DT):
                nc.tensor.matmul(
                    ps[:],
                    ctxT[v][:, qt * P:(qt + 1) * P],
                    wo_sb[v][:, dc * NC:(dc + 1) * NC],
                    start=(v == 0),
                    stop=(v == DT - 1),
                )
            nc.vector.tensor_copy(h[:, dc * NC:(dc + 1) * NC], ps[:])
        # LayerNorm over the free dim via bn_stats/bn_aggr.
        stats = stat_pool.tile(
            [P, n_sub, nc.vector.BN_STATS_DIM], F32, tag="bnstats", name=f"bnst{qt}"
        )
        for i in range(n_sub):
            nc.vector.bn_stats(out=stats[:, i, :], in_=h[:, i * sub:(i + 1) * sub])
        mv = stat_pool.tile([P, nc.vector.BN_AGGR_DIM], F32, tag="bnaggr", name=f"bnag{qt}")
        nc.vector.bn_aggr(out=mv[:], in_=stats[:])
        # rstd = 1/sqrt(var + eps)
        rstd = stat_pool.tile([P, 1], F32, tag="rstd", name=f"rstd{qt}")
        nc.scalar.activation(rstd[:], mv[:, 1:2], AF.Sqrt, bias=eps_sb[:], scale=1.0)
        nc.vector.reciprocal(rstd[:], rstd[:])
        o = o_pool.tile([P, D], F32, tag="o", name=f"o{qt}")
        nc.vector.tensor_scalar(
            out=o[:],
            in0=h[:],
            scalar1=mv[:, 0:1],
            scalar2=rstd[:],
            op0=OP.subtract,
            op1=OP.mult,
        )
        nc.vector.tensor_tensor(o[:], o[:], gamma_sb[:], OP.mult)
        nc.vector.tensor_tensor(o[:], o[:], beta_sb[:], OP.add)
        nc.sync.dma_start(out[qt * P:(qt + 1) * P, :], o[:])
    stat_pool.release()
    o_pool.release()
    h_pool.release()
    ctxT_pool.release()
    gb_pool.release()
    wo_pool.release()
    wv_pool.release()
    xcol_pool.release()


_PROGS: dict = {}


def _build_program(n_iters: int = 1):
    if n_iters not in _PROGS:
        nc = bacc.Bacc(
            "TRN2",
            target_bir_lowering=False,
            debug=False,
            enable_asserts=False,
            num_devices=N_CORES,
        )
        io = {
            "xb": nc.dram_tensor("xb", [S, D], F32R, kind="ExternalInput").ap(),
            "wq": nc.dram_tensor("wq", [D, D], F32R, kind="ExternalInput").ap(),
            "wk": nc.dram_tensor("wk", [D, D], F32R, kind="ExternalInput").ap(),
            "wv": nc.dram_tensor("wv", [D, D], F32R, kind="ExternalInput").ap(),
            "wo": nc.dram_tensor("wo", [D, D], F32R, kind="ExternalInput").ap(),
            "gamma_b": nc.dram_tensor("gamma_b", [P, D], F32, kind="ExternalInput").ap(),
            "beta_b": nc.dram_tensor("beta_b", [P, D], F32, kind="ExternalInput").ap(),
            "out": nc.dram_tensor("out", [NQ, D], F32, kind="ExternalOutput").ap(),
        }
        with tile.TileContext(nc) as tc:
            for _ in range(n_iters):
                with ExitStack() as ctx:
                    _emit(ctx, tc, io)
        nc.compile()
        _PROGS[n_iters] = nc
    return _PROGS[n_iters]


LAST_RESULTS = None


def kernel(x, Wq, Wk, Wv, Wo, ln2_gamma, ln2_beta):
    global LAST_RESULTS
    x = np.ascontiguousarray(np.asarray(x, dtype=np.float32))
    Wq = np.ascontiguousarray(np.asarray(Wq, dtype=np.float32))
    Wk = np.ascontiguousarray(np.asarray(Wk, dtype=np.float32))
    Wv = np.ascontiguousarray(np.asarray(Wv, dtype=np.float32))
    Wo = np.ascontiguousarray(np.asarray(Wo, dtype=np.float32))
    gamma_b = np.ascontiguousarray(
        np.broadcast_to(np.asarray(ln2_gamma, dtype=np.float32), (P, D))
    )
    beta_b = np.ascontiguousarray(
        np.broadcast_to(np.asarray(ln2_beta, dtype=np.float32), (P, D))
    )

    nc = _build_program()
    in_maps = []
    for c in range(N_CORES):
        b, h = c // 2, c % 2
        # Rotate so this core's query rows are rows 0:NQ.
        xb = np.ascontiguousarray(np.roll(x[b], -h * NQ, axis=0))
        in_maps.append(
            {
                "xb": xb,
                "wq": Wq,
                "wk": Wk,
                "wv": Wv,
                "wo": Wo,
                "gamma_b": gamma_b,
                "beta_b": beta_b,
            }
        )
    res = run_bass_kernel_spmd(nc, in_maps, list(range(N_CORES)))
    LAST_RESULTS = res
    out = np.empty((B, S, D), dtype=np.float32)
    for c in range(N_CORES):
        b, h = c // 2, c % 2
        out[b, h * NQ:(h + 1) * NQ] = res.results[c]["out"]
    return out


# revision 12
# speedup vs baseline: 1.1359x; 1.1051x over previous
"""Trainium2 Bass kernel: single-head attention encoder block.

Problem: x[4, 2048, 1024]; q/k/v projections, softmax attention, output
projection, layernorm.  8 NeuronCores, SPMD.

Sharding: core c handles batch b = c // 2 and query-half h = c % 2.
Each core receives its batch's x ROTATED along the sequence axis so that
the core's 1024 query rows always occupy rows 0:1024 (attention is
permutation-invariant over keys as long as K and V share an ordering, so
the rotation only permutes the reduction order).  This keeps the SPMD
program free of per-core constants.

Per-core dataflow (all matmuls in float32r = full PE rate, fp32 storage).
The value path uses associativity:  ctx = A @ (x @ Wv) = (A @ x) @ Wv,
which removes the V projection for the full sequence AND the scratch
round-trip; the Z^T = x^T @ A^T intermediate takes x tiles straight from
DRAM as the stationary operand (no transpose needed), and comes out in
exactly the layout the Wv/Wo projections want.

  xt    = x^T                      (PE transpose, [d partition, s free])
  K^T   = Wk^T @ x                 ([k partition, s free], SBUF resident)
  Q^T   = Wq^T @ x[:1024]          ([k partition, q free], SBUF resident)
  S^T   = K Q^T                    ([s partition, q free] -> exp via ACT)
  den   = ones^T @ exp(S^T)        (column sums via PE broadcast-matmul)
  Z^T   = x^T @ exp(S^T)           (lhsT = x tiles from DRAM, [d, q])
  ctxT  = (Wv^T @ Z^T) * 1/den     ([v, q]; normalization fused in copy)
  h     = ctx @ Wo                 ([q partition, d free])
  out   = layernorm(h) * gamma + beta

SBUF is two LIFO stacks (left/right) so overlapping pool lifetimes nest.
Peak ~185 KB/partition.
"""

from contextlib import ExitStack

import numpy as np

import concourse.bass as bass
import concourse.tile as tile
from concourse import bacc, mybir
from concourse.bass_utils import run_bass_kernel_spmd
from concourse.masks import make_identity

F32 = mybir.dt.float32
F32R = mybir.dt.float32r
AF = mybir.ActivationFunctionType
OP = mybir.AluOpType

B = 4
S = 2048
D = 1024
NQ = 1024  # queries per core
P = 128
DT = D // P   # 8 d-tiles
ST = S // P   # 16 s-tiles
KTN = D // P  # 8 k-tiles
QTN = NQ // P  # 8 q-tiles
NC = 512      # matmul free-dim chunk (one fp32 PSUM bank)
SCN = S // NC   # 4 s-chunks
QCN = NQ // NC  # 2 q-chunks
DCN = D // NC   # 2 d-chunks
N_CORES = 8
SCALE = 1.0 / np.sqrt(np.float32(D))  # 1/32
LN_EPS = 1e-5


def _f32(ap):
    """fp32 view of an f32r AP for DVE/ACT readers (same IEEE bits)."""
    return ap.bitcast(F32)


def _keepalive(nc, tc, aps, out):
    """Read one column of each AP and DMA to out so bacc keeps the work."""
    kp = tc.alloc_tile_pool(name="keep", bufs=1, side="left")
    kt = kp.tile([P, max(len(aps), 1)], F32, tag="keep", name="keept")
    for i, ap in enumerate(aps):
        nc.vector.tensor_copy(kt[:, i:i + 1], ap[:, 0:1].bitcast(F32))
    nc.sync.dma_start(out[0:P, 0:max(len(aps), 1)], kt[:])
    kp.release()


def _emit(ctx: ExitStack, tc: tile.TileContext, io: dict, upto: str = "full"):
    nc = tc.nc
    xb = io["xb"]          # [S, D] f32r
    wq = io["wq"]          # [D, D] f32r
    wk = io["wk"]
    wv = io["wv"]
    wo = io["wo"]
    gamma_b = io["gamma_b"]  # [P, D] f32
    beta_b = io["beta_b"]
    out = io["out"]        # [NQ, D] f32

    const = ctx.enter_context(tc.tile_pool(name="const", bufs=1, side="left"))
    identity_f = const.tile([P, P], F32, tag="identity_f")
    make_identity(nc, identity_f[:])
    identity = const.tile([P, P], F32R, tag="identity")
    nc.vector.tensor_copy(identity[:], identity_f[:])
    ones_f = const.tile([P, P], F32, tag="ones_f")
    nc.vector.memset(ones_f[:], 1.0)
    ones = const.tile([P, P], F32R, tag="ones")
    nc.vector.tensor_copy(ones[:], ones_f[:])
    recip = const.tile([P, NQ], F32, tag="recip")
    eps_sb = const.tile([P, 1], F32, tag="eps")
    nc.vector.memset(eps_sb[:], LN_EPS)

    # PSUM: 8 banks.  mm cycles 4, den 2, transpose 2.
    ps_mm = ctx.enter_context(tc.tile_pool(name="ps_mm", bufs=6, space="PSUM"))
    ps_den = ctx.enter_context(tc.tile_pool(name="ps_den", bufs=1, space="PSUM"))
    ps_tr = ctx.enter_context(tc.tile_pool(name="ps_tr", bufs=1, space="PSUM"))

    # ---- Phases T+K interleaved ----
    # xcol is pre-reserved at the BOTTOM of the right stack: its 8MB of x
    # column tiles (consumed in C1) can then stream during K/Q/S with no
    # released-zone dependency on kt/qt.
    xcol_pool = tc.alloc_tile_pool(name="xcol", bufs=16, side="right")
    kt_pool = tc.alloc_tile_pool(name="ktp", bufs=1, side="right")
    kt_sb = [kt_pool.tile([P, S], F32R, tag=f"kt{k}", name=f"kt{k}") for k in range(KTN)]
    xtb_pool = tc.alloc_tile_pool(name="xtb", bufs=1, side="left")
    xtb = [xtb_pool.tile([P, S], F32R, tag=f"xtb{d}", name=f"xtb{d}") for d in range(DT)]
    wk_pool = tc.alloc_tile_pool(name="wkp", bufs=1, side="left")
    xrow_pool = tc.alloc_tile_pool(name="xrow", bufs=2, side="left")
    wk_sb = [wk_pool.tile([P, D], F32R, tag=f"wkr{d}", name=f"wkr{d}") for d in range(DT)]

    def _transpose_chunk(sc):
        for st in range(4 * sc, 4 * sc + 4):
            xr = xrow_pool.tile([P, D], F32R, tag="xr", name=f"xr{st}")
            nc.sync.dma_start(xr[:], xb[st * P:(st + 1) * P, :])
            for d in range(DT):
                pt = ps_tr.tile([P, P], F32R, tag="tr", name=f"ptT{st}_{d}")
                nc.tensor.transpose(pt[:], xr[:, d * P:(d + 1) * P], identity[:])
                nc.vector.tensor_copy(xtb[d][:, st * P:(st + 1) * P], _f32(pt[:]))

    for sc in range(SCN):
        _transpose_chunk(sc)
        if sc == 0:
            for d in range(DT):
                nc.sync.dma_start(wk_sb[d][:], wk[d * P:(d + 1) * P, :])
        for k in range(KTN):
            ps = ps_mm.tile([P, NC], F32, tag="mm", name=f"psK{k}_{sc}")
            for d in range(DT):
                nc.tensor.matmul(
                    ps[:],
                    wk_sb[d][:, k * P:(k + 1) * P],
                    xtb[d][:, sc * NC:(sc + 1) * NC],
                    start=(d == 0),
                    stop=(d == DT - 1),
                )
            if k % 2 == 0:
                nc.vector.tensor_copy(kt_sb[k][:, sc * NC:(sc + 1) * NC], ps[:])
            else:
                nc.scalar.copy(kt_sb[k][:, sc * NC:(sc + 1) * NC], ps[:])
    xrow_pool.release()
    wk_pool.release()

    if upto == "K":
        _keepalive(nc, tc, [t[:, 0:1] for t in kt_sb] + [t[:, 0:1] for t in xtb], out)
        xtb_pool.release()
        kt_pool.release()
        xcol_pool.release()
        return

    # ---- Phase Q: Q^T = Wq^T @ x[:, :NQ]  ([k, q], resident, right side) ----
    qt_pool = tc.alloc_tile_pool(name="qtp", bufs=1, side="right")
    qt_sb = [qt_pool.tile([P, NQ], F32R, tag=f"qt{k}", name=f"qt{k}") for k in range(KTN)]
    wstream_pool = tc.alloc_tile_pool(name="wstream", bufs=8, side="left")
    for kh in range(2):  # half-row rounds: k in [4*kh, 4*kh+4)
        wq_h = []
        for d in range(DT):
            wt = wstream_pool.tile([P, NC], F32R, tag="wqh", name=f"wqh{kh}_{d}")
            nc.sync.dma_start(wt[:], wq[d * P:(d + 1) * P, kh * NC:(kh + 1) * NC])
            wq_h.append(wt)
        for kk in range(4):
            k = 4 * kh + kk
            for qc in range(QCN):
                ps = ps_mm.tile([P, NC], F32, tag="mm", name=f"psQ{k}_{qc}")
                for d in range(DT):
                    nc.tensor.matmul(
                        ps[:],
                        wq_h[d][:, kk * P:(kk + 1) * P],
                        xtb[d][:, qc * NC:(qc + 1) * NC],
                        start=(d == 0),
                        stop=(d == DT - 1),
                    )
                if k % 2 == 0:
                    nc.vector.tensor_copy(qt_sb[k][:, qc * NC:(qc + 1) * NC], ps[:])
                else:
                    nc.scalar.copy(qt_sb[k][:, qc * NC:(qc + 1) * NC], ps[:])
    wstream_pool.release()
    xtb_pool.release()

    if upto == "Q":
        _keepalive(nc, tc, [t[:, 0:1] for t in kt_sb] + [t[:, 0:1] for t in qt_sb], out)
        qt_pool.release()
        kt_pool.release()
        xcol_pool.release()
        return

    # ---- Phase S: scores^T -> exp (UNNORMALIZED), denominators ----
    at_pool = tc.alloc_tile_pool(name="atp", bufs=1, side="left")
    at_sb = [at_pool.tile([P, NQ], F32R, tag=f"at{st}", name=f"at{st}") for st in range(ST)]
    den_pool = tc.alloc_tile_pool(name="denp", bufs=2, side="left")
    for qc in range(QCN):
        dsb = den_pool.tile([P, NC], F32, tag="densb", name=f"densb{qc}")
        nc.vector.memset(dsb[:], 0.0)
        for st in range(ST):
            ps = ps_mm.tile([P, NC], F32, tag="mm", name=f"psS{qc}_{st}")
            for k in range(KTN):
                nc.tensor.matmul(
                    ps[:],
                    kt_sb[k][:, st * P:(st + 1) * P],
                    qt_sb[k][:, qc * NC:(qc + 1) * NC],
                    start=(k == 0),
                    stop=(k == KTN - 1),
                )
            # attn = exp(scores / sqrt(dk)); max-subtraction is unnecessary
            # here (scores are O(1) by construction) and softmax is
            # shift-invariant, so this matches the reference.
            nc.scalar.activation(
                at_sb[st][:, qc * NC:(qc + 1) * NC], ps[:], AF.Exp, scale=float(SCALE)
            )
            nc.vector.tensor_tensor(
                dsb[:], dsb[:], _f32(at_sb[st][:, qc * NC:(qc + 1) * NC]), OP.add
            )
        # Column sums replicated to all 128 partitions: ones[128,128]^T @ dsb.
        dsr = den_pool.tile([P, NC], F32R, tag="densr", name=f"densr{qc}")
        nc.vector.tensor_copy(dsr[:], dsb[:])
        dps = ps_den.tile([P, NC], F32, tag="den", name=f"dps{qc}")
        nc.tensor.matmul(dps[:], ones[:], dsr[:], start=True, stop=True)
        nc.vector.reciprocal(recip[:, qc * NC:(qc + 1) * NC], dps[:])
    den_pool.release()
    qt_pool.release()
    kt_pool.release()

    if upto == "S":
        _keepalive(nc, tc, [t[:, 0:1] for t in at_sb] + [recip[:, 0:1]], out)
        at_pool.release()
        xcol_pool.release()
        return

    # ---- Phase C1: Z^T = x^T @ exp(S^T)  ([d, q]; x tiles from DRAM) ----
    # Wv/Wo prefetch (right side, on the kt/qt zone): their DMA overlaps
    # the C1 matmuls; wv lands first (needed at C2, wo only at O).
    wv_pool = tc.alloc_tile_pool(name="wvp", bufs=1, side="right")
    wv_sb = [wv_pool.tile([P, D], F32R, tag=f"wv{d}", name=f"wv{d}") for d in range(DT)]
    for d in range(DT):
        nc.sync.dma_start(wv_sb[d][:], wv[d * P:(d + 1) * P, :])
    wo_pool = tc.alloc_tile_pool(name="wop", bufs=1, side="right")
    wo_sb = [wo_pool.tile([P, D], F32R, tag=f"wo{v}", name=f"wo{v}") for v in range(DT)]
    for v in range(DT):
        nc.sync.dma_start(wo_sb[v][:], wo[v * P:(v + 1) * P, :])
    zt_pool = tc.alloc_tile_pool(name="ztp", bufs=1, side="right")
    zt_sb = [zt_pool.tile([P, NQ], F32R, tag=f"zt{d}", name=f"zt{d}") for d in range(DT)]
    for dp in range(DT // 2):
        xc = []
        for st in range(ST):
            t = xcol_pool.tile([P, 2 * P], F32R, tag="xc", name=f"xc{dp}_{st}")
            nc.sync.dma_start(
                t[:], xb[st * P:(st + 1) * P, dp * 2 * P:(dp + 1) * 2 * P])
            xc.append(t)
        for dh in range(2):
            d = 2 * dp + dh
            for qc in range(QCN):
                ps = ps_mm.tile([P, NC], F32, tag="mm", name=f"psZ{d}_{qc}")
                for st in range(ST):
                    nc.tensor.matmul(
                        ps[:],
                        xc[st][:, dh * P:(dh + 1) * P],
                        at_sb[st][:, qc * NC:(qc + 1) * NC],
                        start=(st == 0),
                        stop=(st == ST - 1),
                    )
                if d % 2 == 0:
                    nc.vector.tensor_copy(zt_sb[d][:, qc * NC:(qc + 1) * NC], ps[:])
                else:
                    nc.scalar.copy(zt_sb[d][:, qc * NC:(qc + 1) * NC], ps[:])
    at_pool.release()

    if upto == "C1":
        _keepalive(
            nc, tc,
            [t[:, 0:1] for t in zt_sb] + [t[:, 0:1] for t in wv_sb]
            + [t[:, 0:1] for t in wo_sb] + [recip[:, 0:1]], out)
        zt_pool.release()
        wo_pool.release()
        wv_pool.release()
        xcol_pool.release()
        return

    # ---- Phase C2: ctxT = (Wv^T @ Z^T) * 1/den  ([v, q]) ----
    gb_pool = tc.alloc_tile_pool(name="gbp", bufs=1, side="left")
    gamma_sb = gb_pool.tile([P, D], F32, tag="gamma", name="gamma_sb")
    nc.sync.dma_start(gamma_sb[:], gamma_b[:])
    beta_sb = gb_pool.tile([P, D], F32, tag="beta", name="beta_sb")
    nc.sync.dma_start(beta_sb[:], beta_b[:])
    ctxT_pool = tc.alloc_tile_pool(name="ctxTp", bufs=1, side="left")
    ctxT = [ctxT_pool.tile([P, NQ], F32R, tag=f"cxT{v}", name=f"cxT{v}") for v in range(DT)]
    for vt in range(DT):
        for qc in range(QCN):
            ps = ps_mm.tile([P, NC], F32, tag="mm", name=f"psC{vt}_{qc}")
            for d in range(DT):
                nc.tensor.matmul(
                    ps[:],
                    wv_sb[d][:, vt * P:(vt + 1) * P],
                    zt_sb[d][:, qc * NC:(qc + 1) * NC],
                    start=(d == 0),
                    stop=(d == DT - 1),
                )
            # normalization fused into the PSUM drain
            nc.vector.tensor_tensor(
                ctxT[vt][:, qc * NC:(qc + 1) * NC],
                ps[:],
                recip[:, qc * NC:(qc + 1) * NC],
                OP.mult,
            )
    zt_pool.release()

    if upto == "C2":
        _keepalive(
            nc, tc,
            [t[:, 0:1] for t in ctxT] + [t[:, 0:1] for t in wo_sb]
            + [gamma_sb[:, 0:1], beta_sb[:, 0:1]], out)
        ctxT_pool.release()
        gb_pool.release()
        wo_pool.release()
        wv_pool.release()
        xcol_pool.release()
        return

    # ---- Phase O: h = ctx @ Wo, layernorm, store ----
    h_pool = tc.alloc_tile_pool(name="hp", bufs=2, side="left")
    o_pool = tc.alloc_tile_pool(name="op", bufs=2, side="left")
    stat_pool = tc.alloc_tile_pool(name="statp", bufs=4, side="left")
    BN_FMAX = nc.vector.BN_STATS_FMAX
    n_sub = (D + BN_FMAX - 1) // BN_FMAX
    sub = D // n_sub
    for qt in range(QTN):
        h = h_pool.tile([P, D], F32, tag="h", name=f"h{qt}")
        for dc in range(DCN):
            ps = ps_mm.tile([P, NC], F32, tag="mm", name=f"psO{qt}_{dc}")
            for v in range(DT):
                nc.tensor.matmul(
                    ps[:],
                    ctxT[v][:, qt * P:(qt + 1) * P],
                    wo_sb[v][:, dc * NC:(dc + 1) * NC],
                    start=(v == 0),
                    stop=(v == DT - 1),
                )
            if dc % 2 == 0:
                nc.vector.tensor_copy(h[:, dc * NC:(dc + 1) * NC], ps[:])
            else:
                nc.scalar.copy(h[:, dc * NC:(dc + 1) * NC], ps[:])
        # LayerNorm over the free dim via bn_stats/bn_aggr.
        stats = stat_pool.tile(
            [P, n_sub, nc.vector.BN_STATS_DIM], F32, tag="bnstats", name=f"bnst{qt}"
        )
        for i in range(n_sub):
            nc.vector.bn_stats(out=stats[:, i, :], in_=h[:, i * sub:(i + 1) * sub])
        mv = stat_pool.tile([P, nc.vector.BN_AGGR_DIM], F32, tag="bnaggr", name=f"bnag{qt}")
        nc.vector.bn_aggr(out=mv[:], in_=stats[:])
        # rstd = 1/sqrt(var + eps)
        rstd = stat_pool.tile([P, 1], F32, tag="rstd", name=f"rstd{qt}")
        nc.scalar.activation(rstd[:], mv[:, 1:2], AF.Sqrt, bias=eps_sb[:], scale=1.0)
        nc.vector.reciprocal(rstd[:], rstd[:])
        o = o_pool.tile([P, D], F32, tag="o", name=f"o{qt}")
        nc.vector.tensor_scalar(
            out=o[:],
            in0=h[:],
            scalar1=mv[:, 0:1],
            scalar2=rstd[:],
            op0=OP.subtract,
            op1=OP.mult,
        )
        nc.vector.tensor_tensor(o[:], o[:], gamma_sb[:], OP.mult)
        nc.vector.tensor_tensor(o[:], o[:], beta_sb[:], OP.add)
        nc.sync.dma_start(out[qt * P:(qt + 1) * P, :], o[:])
    stat_pool.release()
    o_pool.release()
    h_pool.release()
    ctxT_pool.release()
    gb_pool.release()
    wo_pool.release()
    wv_pool.release()
    xcol_pool.release()


_PROGS: dict = {}


def _build_program(n_iters: int = 1, upto: str = "full"):
    key = (n_iters, upto)
    if key not in _PROGS:
        nc = bacc.Bacc(
            "TRN2",
            target_bir_lowering=False,
            debug=False,
            enable_asserts=False,
            num_devices=N_CORES,
        )
        io = {
            "xb": nc.dram_tensor("xb", [S, D], F32R, kind="ExternalInput").ap(),
            "wq": nc.dram_tensor("wq", [D, D], F32R, kind="ExternalInput").ap(),
            "wk": nc.dram_tensor("wk", [D, D], F32R, kind="ExternalInput").ap(),
            "wv": nc.dram_tensor("wv", [D, D], F32R, kind="ExternalInput").ap(),
            "wo": nc.dram_tensor("wo", [D, D], F32R, kind="ExternalInput").ap(),
            "gamma_b": nc.dram_tensor("gamma_b", [P, D], F32, kind="ExternalInput").ap(),
            "beta_b": nc.dram_tensor("beta_b", [P, D], F32, kind="ExternalInput").ap(),
            "out": nc.dram_tensor("out", [NQ, D], F32, kind="ExternalOutput").ap(),
        }
        with tile.TileContext(nc) as tc:
            for _ in range(n_iters):
                with ExitStack() as ctx:
                    _emit(ctx, tc, io, upto)
        nc.compile()
        _PROGS[key] = nc
    return _PROGS[key]


LAST_RESULTS = None


def kernel(x, Wq, Wk, Wv, Wo, ln2_gamma, ln2_beta):
    global LAST_RESULTS
    x = np.ascontiguousarray(np.asarray(x, dtype=np.float32))
    Wq = np.ascontiguousarray(np.asarray(Wq, dtype=np.float32))
    Wk = np.ascontiguousarray(np.asarray(Wk, dtype=np.float32))
    Wv = np.ascontiguousarray(np.asarray(Wv, dtype=np.float32))
    Wo = np.ascontiguousarray(np.asarray(Wo, dtype=np.float32))
    gamma_b = np.ascontiguousarray(
        np.broadcast_to(np.asarray(ln2_gamma, dtype=np.float32), (P, D))
    )
    beta_b = np.ascontiguousarray(
        np.broadcast_to(np.asarray(ln2_beta, dtype=np.float32), (P, D))
    )

    nc = _build_program()
    in_maps = []
    for c in range(N_CORES):
        b, h = c // 2, c % 2
        # Rotate so this core's query rows are rows 0:NQ.
        xb = np.ascontiguousarray(np.roll(x[b], -h * NQ, axis=0))
        in_maps.append(
            {
                "xb": xb,
                "wq": Wq,
                "wk": Wk,
                "wv": Wv,
                "wo": Wo,
                "gamma_b": gamma_b,
                "beta_b": beta_b,
            }
        )
    res = run_bass_kernel_spmd(nc, in_maps, list(range(N_CORES)))
    LAST_RESULTS = res
    out = np.empty((B, S, D), dtype=np.float32)
    for c in range(N_CORES):
        b, h = c // 2, c % 2
        out[b, h * NQ:(h + 1) * NQ] = res.results[c]["out"]
    return out


# revision 14
# speedup vs baseline: 1.3637x; 1.2006x over previous
"""Trainium2 Bass kernel: single-head attention encoder block.

Problem: x[4, 2048, 1024]; q/k/v projections, softmax attention, output
projection, layernorm.  8 NeuronCores, SPMD.

Sharding: core c handles batch b = c // 2 and query-half h = c % 2.
Each core receives its batch's x ROTATED along the sequence axis so that
the core's 1024 query rows always occupy rows 0:1024 (attention is
permutation-invariant over keys as long as K and V share an ordering, so
the rotation only permutes the reduction order).  This keeps the SPMD
program free of per-core constants.

Per-core dataflow (all matmuls in float32r = full PE rate, fp32 storage).
The value path uses associativity:  ctx = A @ (x @ Wv) = (A @ x) @ Wv,
which removes the V projection for the full sequence AND the scratch
round-trip; the Z^T = x^T @ A^T intermediate takes x tiles straight from
DRAM as the stationary operand (no transpose needed), and comes out in
exactly the layout the Wv/Wo projections want.

  xt    = x^T                      (PE transpose, [d partition, s free])
  K^T   = Wk^T @ x                 ([k partition, s free], SBUF resident)
  Q^T   = Wq^T @ x[:1024]          ([k partition, q free], SBUF resident)
  S^T   = K Q^T                    ([s partition, q free] -> exp via ACT)
  den   = ones^T @ exp(S^T)        (column sums via PE broadcast-matmul)
  Z^T   = x^T @ exp(S^T)           (lhsT = x tiles from DRAM, [d, q])
  ctxT  = (Wv^T @ Z^T) * 1/den     ([v, q]; normalization fused in copy)
  h     = ctx @ Wo                 ([q partition, d free])
  out   = layernorm(h) * gamma + beta

SBUF is two LIFO stacks (left/right) so overlapping pool lifetimes nest.
Peak ~185 KB/partition.
"""

from contextlib import ExitStack

import numpy as np

import concourse.bass as bass
import concourse.tile as tile
from concourse import bacc, mybir
from concourse.bass_utils import run_bass_kernel_spmd
from concourse.masks import make_identity

F32 = mybir.dt.float32
F32R = mybir.dt.float32r
AF = mybir.ActivationFunctionType
OP = mybir.AluOpType

B = 4
S = 2048
D = 1024
NQ = 1024  # queries per core
P = 128
DT = D // P   # 8 d-tiles
ST = S // P   # 16 s-tiles
KTN = D // P  # 8 k-tiles
QTN = NQ // P  # 8 q-tiles
NC = 512      # matmul free-dim chunk (one fp32 PSUM bank)
SCN = S // NC   # 4 s-chunks
QCN = NQ // NC  # 2 q-chunks
DCN = D // NC   # 2 d-chunks
N_CORES = 8
SCALE = 1.0 / np.sqrt(np.float32(D))  # 1/32
LN_EPS = 1e-5


def _f32(ap):
    """fp32 view of an f32r AP for DVE/ACT readers (same IEEE bits)."""
    return ap.bitcast(F32)


def _keepalive(nc, tc, aps, out):
    """Read one column of each AP and DMA to out so bacc keeps the work."""
    kp = tc.alloc_tile_pool(name="keep", bufs=1, side="left")
    kt = kp.tile([P, max(len(aps), 1)], F32, tag="keep", name="keept")
    for i, ap in enumerate(aps):
        nc.vector.tensor_copy(kt[:, i:i + 1], ap[:, 0:1].bitcast(F32))
    nc.sync.dma_start(out[0:P, 0:max(len(aps), 1)], kt[:])
    kp.release()


def _emit(ctx: ExitStack, tc: tile.TileContext, io: dict, upto: str = "full"):
    nc = tc.nc
    xb = io["xb"]          # [S, D] f32r
    wq = io["wq"]          # [D, D] f32r
    wk = io["wk"]
    wv = io["wv"]
    wo = io["wo"]
    gamma_b = io["gamma_b"]  # [P, D] f32
    beta_b = io["beta_b"]
    out = io["out"]        # [NQ, D] f32

    const = ctx.enter_context(tc.tile_pool(name="const", bufs=1, side="left"))
    identity_f = const.tile([P, P], F32, tag="identity_f")
    make_identity(nc, identity_f[:])
    identity = const.tile([P, P], F32R, tag="identity")
    nc.vector.tensor_copy(identity[:], identity_f[:])
    ones_f = const.tile([P, P], F32, tag="ones_f")
    nc.vector.memset(ones_f[:], 1.0)
    ones = const.tile([P, P], F32R, tag="ones")
    nc.vector.tensor_copy(ones[:], ones_f[:])
    recip = const.tile([P, NQ], F32, tag="recip")
    eps_sb = const.tile([P, 1], F32, tag="eps")
    nc.vector.memset(eps_sb[:], LN_EPS)

    # PSUM: 8 banks.  mm cycles 4, den 2, transpose 2.
    ps_mm = ctx.enter_context(tc.tile_pool(name="ps_mm", bufs=5, space="PSUM"))
    ps_tr = ctx.enter_context(tc.tile_pool(name="ps_tr", bufs=3, space="PSUM"))
    ps_den = ps_tr  # den psum tiles reuse the tr slots (tr idle during S)

    # ---- Phases T+K interleaved ----
    # xcol is pre-reserved at the BOTTOM of the right stack: its 8MB of x
    # column tiles (consumed in C1) can then stream during K/Q/S with no
    # released-zone dependency on kt/qt.
    xcol_pool = tc.alloc_tile_pool(name="xcol", bufs=16, side="right")
    kt_pool = tc.alloc_tile_pool(name="ktp", bufs=1, side="right")
    kt_sb = [kt_pool.tile([P, S], F32R, tag=f"kt{k}", name=f"kt{k}") for k in range(KTN)]
    xtb_pool = tc.alloc_tile_pool(name="xtb", bufs=1, side="left")
    xtb = [xtb_pool.tile([P, S], F32R, tag=f"xtb{d}", name=f"xtb{d}") for d in range(DT)]
    wstream_pool = tc.alloc_tile_pool(name="wstream", bufs=8, side="left")
    wq_h0: list = []
    wk_pool = tc.alloc_tile_pool(name="wkp", bufs=1, side="left")
    xrow_pool = tc.alloc_tile_pool(name="xrow", bufs=2, side="left")
    wk_sb = [wk_pool.tile([P, D], F32R, tag=f"wkr{d}", name=f"wkr{d}") for d in range(DT)]

    def _transpose_chunk(sc):
        for st in range(4 * sc, 4 * sc + 4):
            xr = xrow_pool.tile([P, D], F32R, tag="xr", name=f"xr{st}")
            nc.sync.dma_start(xr[:], xb[st * P:(st + 1) * P, :])
            for d in range(DT):
                pt = ps_tr.tile([P, P], F32R, tag="tr", name=f"ptT{st}_{d}")
                nc.tensor.transpose(pt[:], xr[:, d * P:(d + 1) * P], identity[:])
                if d % 2 == 0:
                    nc.vector.tensor_copy(
                        xtb[d][:, st * P:(st + 1) * P], _f32(pt[:]))
                else:
                    nc.scalar.copy(xtb[d][:, st * P:(st + 1) * P], _f32(pt[:]))

    for sc in range(SCN):
        _transpose_chunk(sc)
        if sc == 0:
            for d in range(DT):
                nc.sync.dma_start(wk_sb[d][:], wk[d * P:(d + 1) * P, :])
        if sc == 1:
            # wq first-half prefetch: lands well before phase Q needs it
            for d in range(DT):
                wt = wstream_pool.tile([P, NC], F32R, tag="wqh", name=f"wqh0_{d}")
                nc.sync.dma_start(wt[:], wq[d * P:(d + 1) * P, 0:NC])
                wq_h0.append(wt)
        for k in range(KTN):
            ps = ps_mm.tile([P, NC], F32, tag="mm", name=f"psK{k}_{sc}")
            for d in range(DT):
                nc.tensor.matmul(
                    ps[:],
                    wk_sb[d][:, k * P:(k + 1) * P],
                    xtb[d][:, sc * NC:(sc + 1) * NC],
                    start=(d == 0),
                    stop=(d == DT - 1),
                )
            if k % 2 == 0:
                nc.vector.tensor_copy(kt_sb[k][:, sc * NC:(sc + 1) * NC], ps[:])
            else:
                nc.scalar.copy(kt_sb[k][:, sc * NC:(sc + 1) * NC], ps[:])
    xrow_pool.release()
    wk_pool.release()

    if upto == "K":
        _keepalive(nc, tc, [t[:, 0:1] for t in kt_sb] + [t[:, 0:1] for t in xtb], out)
        xtb_pool.release()
        kt_pool.release()
        xcol_pool.release()
        return

    # ---- Phase Q: Q^T = Wq^T @ x[:, :NQ]  ([k, q], resident, right side) ----
    qt_pool = tc.alloc_tile_pool(name="qtp", bufs=1, side="right")
    qt_sb = [qt_pool.tile([P, NQ], F32R, tag=f"qt{k}", name=f"qt{k}") for k in range(KTN)]
    for kh in range(2):  # half-row rounds: k in [4*kh, 4*kh+4)
        if kh == 0:
            wq_h = wq_h0
        else:
            wq_h = []
            for d in range(DT):
                wt = wstream_pool.tile([P, NC], F32R, tag="wqh", name=f"wqh1_{d}")
                nc.sync.dma_start(wt[:], wq[d * P:(d + 1) * P, NC:2 * NC])
                wq_h.append(wt)
        for kk in range(4):
            k = 4 * kh + kk
            for qc in range(QCN):
                ps = ps_mm.tile([P, NC], F32, tag="mm", name=f"psQ{k}_{qc}")
                for d in range(DT):
                    nc.tensor.matmul(
                        ps[:],
                        wq_h[d][:, kk * P:(kk + 1) * P],
                        xtb[d][:, qc * NC:(qc + 1) * NC],
                        start=(d == 0),
                        stop=(d == DT - 1),
                    )
                if k % 2 == 0:
                    nc.vector.tensor_copy(qt_sb[k][:, qc * NC:(qc + 1) * NC], ps[:])
                else:
                    nc.scalar.copy(qt_sb[k][:, qc * NC:(qc + 1) * NC], ps[:])
    wstream_pool.release()
    xtb_pool.release()

    if upto == "Q":
        _keepalive(nc, tc, [t[:, 0:1] for t in kt_sb] + [t[:, 0:1] for t in qt_sb], out)
        qt_pool.release()
        kt_pool.release()
        xcol_pool.release()
        return

    # ---- Phase S: scores^T -> exp (UNNORMALIZED), denominators ----
    at_pool = tc.alloc_tile_pool(name="atp", bufs=1, side="left")
    at_sb = [at_pool.tile([P, NQ], F32R, tag=f"at{st}", name=f"at{st}") for st in range(ST)]
    den_pool = tc.alloc_tile_pool(name="denp", bufs=2, side="left")
    for qc in range(QCN):
        dsb = den_pool.tile([P, NC], F32, tag="densb", name=f"densb{qc}")
        nc.vector.memset(dsb[:], 0.0)
        for st in range(ST):
            ps = ps_mm.tile([P, NC], F32, tag="mm", name=f"psS{qc}_{st}")
            for k in range(KTN):
                nc.tensor.matmul(
                    ps[:],
                    kt_sb[k][:, st * P:(st + 1) * P],
                    qt_sb[k][:, qc * NC:(qc + 1) * NC],
                    start=(k == 0),
                    stop=(k == KTN - 1),
                )
            # attn = exp(scores / sqrt(dk)); max-subtraction is unnecessary
            # here (scores are O(1) by construction) and softmax is
            # shift-invariant, so this matches the reference.
            nc.scalar.activation(
                at_sb[st][:, qc * NC:(qc + 1) * NC], ps[:], AF.Exp, scale=float(SCALE)
            )
            nc.vector.tensor_tensor(
                dsb[:], dsb[:], _f32(at_sb[st][:, qc * NC:(qc + 1) * NC]), OP.add
            )
        # Column sums replicated to all 128 partitions: ones[128,128]^T @ dsb.
        dsr = den_pool.tile([P, NC], F32R, tag="densr", name=f"densr{qc}")
        nc.vector.tensor_copy(dsr[:], dsb[:])
        dps = ps_den.tile([P, NC], F32, tag="tr", name=f"dps{qc}")
        nc.tensor.matmul(dps[:], ones[:], dsr[:], start=True, stop=True)
        nc.vector.reciprocal(recip[:, qc * NC:(qc + 1) * NC], dps[:])
    den_pool.release()
    qt_pool.release()
    kt_pool.release()

    if upto == "S":
        _keepalive(nc, tc, [t[:, 0:1] for t in at_sb] + [recip[:, 0:1]], out)
        at_pool.release()
        xcol_pool.release()
        return

    # ---- Phase C1: Z^T = x^T @ exp(S^T)  ([d, q]; x tiles from DRAM) ----
    # Wv/Wo prefetch (right side, on the kt/qt zone): their DMA overlaps
    # the C1 matmuls; wv lands first (needed at C2, wo only at O).
    wv_pool = tc.alloc_tile_pool(name="wvp", bufs=1, side="right")
    wv_sb = [wv_pool.tile([P, D], F32R, tag=f"wv{d}", name=f"wv{d}") for d in range(DT)]
    for d in range(DT):
        nc.sync.dma_start(wv_sb[d][:], wv[d * P:(d + 1) * P, :])
    wo_pool = tc.alloc_tile_pool(name="wop", bufs=1, side="right")
    wo_sb = [wo_pool.tile([P, D], F32R, tag=f"wo{v}", name=f"wo{v}") for v in range(DT)]
    for v in range(DT):
        nc.sync.dma_start(wo_sb[v][:], wo[v * P:(v + 1) * P, :])
    zt_pool = tc.alloc_tile_pool(name="ztp", bufs=1, side="right")
    zt_sb = [zt_pool.tile([P, NQ], F32R, tag=f"zt{d}", name=f"zt{d}") for d in range(DT)]
    for dp in range(DT // 2):
        xc = []
        for st in range(ST):
            t = xcol_pool.tile([P, 2 * P], F32R, tag="xc", name=f"xc{dp}_{st}")
            nc.sync.dma_start(
                t[:], xb[st * P:(st + 1) * P, dp * 2 * P:(dp + 1) * 2 * P])
            xc.append(t)
        for dh in range(2):
            d = 2 * dp + dh
            for qc in range(QCN):
                ps = ps_mm.tile([P, NC], F32, tag="mm", name=f"psZ{d}_{qc}")
                for st in range(ST):
                    nc.tensor.matmul(
                        ps[:],
                        xc[st][:, dh * P:(dh + 1) * P],
                        at_sb[st][:, qc * NC:(qc + 1) * NC],
                        start=(st == 0),
                        stop=(st == ST - 1),
                    )
                if d % 2 == 0:
                    nc.vector.tensor_copy(zt_sb[d][:, qc * NC:(qc + 1) * NC], ps[:])
                else:
                    nc.scalar.copy(zt_sb[d][:, qc * NC:(qc + 1) * NC], ps[:])
    at_pool.release()

    if upto == "C1":
        _keepalive(
            nc, tc,
            [t[:, 0:1] for t in zt_sb] + [t[:, 0:1] for t in wv_sb]
            + [t[:, 0:1] for t in wo_sb] + [recip[:, 0:1]], out)
        zt_pool.release()
        wo_pool.release()
        wv_pool.release()
        xcol_pool.release()
        return

    # ---- Phase C2: ctxT = (Wv^T @ Z^T) * 1/den  ([v, q]) ----
    gb_pool = tc.alloc_tile_pool(name="gbp", bufs=1, side="left")
    gamma_sb = gb_pool.tile([P, D], F32, tag="gamma", name="gamma_sb")
    nc.sync.dma_start(gamma_sb[:], gamma_b[:])
    beta_sb = gb_pool.tile([P, D], F32, tag="beta", name="beta_sb")
    nc.sync.dma_start(beta_sb[:], beta_b[:])
    ctxT_pool = tc.alloc_tile_pool(name="ctxTp", bufs=1, side="left")
    ctxT = [ctxT_pool.tile([P, NQ], F32R, tag=f"cxT{v}", name=f"cxT{v}") for v in range(DT)]
    for vt in range(DT):
        for qc in range(QCN):
            ps = ps_mm.tile([P, NC], F32, tag="mm", name=f"psC{vt}_{qc}")
            for d in range(DT):
                nc.tensor.matmul(
                    ps[:],
                    wv_sb[d][:, vt * P:(vt + 1) * P],
                    zt_sb[d][:, qc * NC:(qc + 1) * NC],
                    start=(d == 0),
                    stop=(d == DT - 1),
                )
            # normalization fused into the PSUM drain
            nc.vector.tensor_tensor(
                ctxT[vt][:, qc * NC:(qc + 1) * NC],
                ps[:],
                recip[:, qc * NC:(qc + 1) * NC],
                OP.mult,
            )
    zt_pool.release()

    if upto == "C2":
        _keepalive(
            nc, tc,
            [t[:, 0:1] for t in ctxT] + [t[:, 0:1] for t in wo_sb]
            + [gamma_sb[:, 0:1], beta_sb[:, 0:1]], out)
        ctxT_pool.release()
        gb_pool.release()
        wo_pool.release()
        wv_pool.release()
        xcol_pool.release()
        return

    # ---- Phase O: h = ctx @ Wo, layernorm, store ----
    h_pool = tc.alloc_tile_pool(name="hp", bufs=2, side="left")
    o_pool = tc.alloc_tile_pool(name="op", bufs=2, side="left")
    stat_pool = tc.alloc_tile_pool(name="statp", bufs=4, side="left")
    BN_FMAX = nc.vector.BN_STATS_FMAX
    n_sub = (D + BN_FMAX - 1) // BN_FMAX
    sub = D // n_sub
    for qt in range(QTN):
        h = h_pool.tile([P, D], F32, tag="h", name=f"h{qt}")
        for dc in range(DCN):
            ps = ps_mm.tile([P, NC], F32, tag="mm", name=f"psO{qt}_{dc}")
            for v in range(DT):
                nc.tensor.matmul(
                    ps[:],
                    ctxT[v][:, qt * P:(qt + 1) * P],
                    wo_sb[v][:, dc * NC:(dc + 1) * NC],
                    start=(v == 0),
                    stop=(v == DT - 1),
                )
            if dc % 2 == 0:
                nc.vector.tensor_copy(h[:, dc * NC:(dc + 1) * NC], ps[:])
            else:
                nc.scalar.copy(h[:, dc * NC:(dc + 1) * NC], ps[:])
        # LayerNorm over the free dim via bn_stats/bn_aggr.
        stats = stat_pool.tile(
            [P, n_sub, nc.vector.BN_STATS_DIM], F32, tag="bnstats", name=f"bnst{qt}"
        )
        for i in range(n_sub):
            nc.vector.bn_stats(out=stats[:, i, :], in_=h[:, i * sub:(i + 1) * sub])
        mv = stat_pool.tile([P, nc.vector.BN_AGGR_DIM], F32, tag="bnaggr", name=f"bnag{qt}")
        nc.vector.bn_aggr(out=mv[:], in_=stats[:])
        # rstd = 1/sqrt(var + eps)
        rstd = stat_pool.tile([P, 1], F32, tag="rstd", name=f"rstd{qt}")
        nc.scalar.activation(rstd[:], mv[:, 1:2], AF.Sqrt, bias=eps_sb[:], scale=1.0)
        nc.vector.reciprocal(rstd[:], rstd[:])
        o = o_pool.tile([P, D], F32, tag="o", name=f"o{qt}")
        nc.vector.tensor_scalar(
            out=o[:],
            in0=h[:],
            scalar1=mv[:, 0:1],
            scalar2=rstd[:],
            op0=OP.subtract,
            op1=OP.mult,
        )
        nc.vector.tensor_tensor(o[:], o[:], gamma_sb[:], OP.mult)
        nc.vector.tensor_tensor(o[:], o[:], beta_sb[:], OP.add)
        nc.sync.dma_start(out[qt * P:(qt + 1) * P, :], o[:])
    stat_pool.release()
    o_pool.release()
    h_pool.release()
    ctxT_pool.release()
    gb_pool.release()
    wo_pool.release()
    wv_pool.release()
    xcol_pool.release()


_PROGS: dict = {}


def _build_program(n_iters: int = 1, upto: str = "full"):
    key = (n_iters, upto)
    if key not in _PROGS:
        nc = bacc.Bacc(
            "TRN2",
            target_bir_lowering=False,
            debug=False,
            enable_asserts=False,
            num_devices=N_CORES,
        )
        io = {
            "xb": nc.dram_tensor("xb", [S, D], F32R, kind="ExternalInput").ap(),
            "wq": nc.dram_tensor("wq", [D, D], F32R, kind="ExternalInput").ap(),
            "wk": nc.dram_tensor("wk", [D, D], F32R, kind="ExternalInput").ap(),
            "wv": nc.dram_tensor("wv", [D, D], F32R, kind="ExternalInput").ap(),
            "wo": nc.dram_tensor("wo", [D, D], F32R, kind="ExternalInput").ap(),
            "gamma_b": nc.dram_tensor("gamma_b", [P, D], F32, kind="ExternalInput").ap(),
            "beta_b": nc.dram_tensor("beta_b", [P, D], F32, kind="ExternalInput").ap(),
            "out": nc.dram_tensor("out", [NQ, D], F32, kind="ExternalOutput").ap(),
        }
        with tile.TileContext(nc) as tc:
            for _ in range(n_iters):
                with ExitStack() as ctx:
                    _emit(ctx, tc, io, upto)
        nc.compile()
        _PROGS[key] = nc
    return _PROGS[key]


LAST_RESULTS = None


def kernel(x, Wq, Wk, Wv, Wo, ln2_gamma, ln2_beta):
    global LAST_RESULTS
    x = np.ascontiguousarray(np.asarray(x, dtype=np.float32))
    Wq = np.ascontiguousarray(np.asarray(Wq, dtype=np.float32))
    Wk = np.ascontiguousarray(np.asarray(Wk, dtype=np.float32))
    Wv = np.ascontiguousarray(np.asarray(Wv, dtype=np.float32))
    Wo = np.ascontiguousarray(np.asarray(Wo, dtype=np.float32))
    gamma_b = np.ascontiguousarray(
        np.broadcast_to(np.asarray(ln2_gamma, dtype=np.float32), (P, D))
    )
    beta_b = np.ascontiguousarray(
        np.broadcast_to(np.asarray(ln2_beta, dtype=np.float32), (P, D))
    )

    nc = _build_program()
    in_maps = []
    for c in range(N_CORES):
        b, h = c // 2, c % 2
        # Rotate so this core's query rows are rows 0:NQ.
        xb = np.ascontiguousarray(np.roll(x[b], -h * NQ, axis=0))
        in_maps.append(
            {
                "xb": xb,
                "wq": Wq,
                "wk": Wk,
                "wv": Wv,
                "wo": Wo,
                "gamma_b": gamma_b,
                "beta_b": beta_b,
            }
        )
    res = run_bass_kernel_spmd(nc, in_maps, list(range(N_CORES)))
    LAST_RESULTS = res
    out = np.empty((B, S, D), dtype=np.float32)
    for c in range(N_CORES):
        b, h = c // 2, c % 2
        out[b, h * NQ:(h + 1) * NQ] = res.results[c]["out"]
    return out
